# revision 11
# baseline (speedup 1.0000x reference)
"""AttentivePooling Trainium2 kernel (8 NeuronCores, SPMD).

Math (per graph g):  pooled[g] = sum_{n in g} softmax_g(s)_n * x[n]
with s_n = tanh(x W1 + b1) W2 + b2.  Since tanh bounds |s| <= ||W2||_1 + |b2|
(~9 for these inputs), the segment-max subtraction in the reference is
unnecessary: we accumulate  num[g] = sum exp(s_n - SHIFT) x_n  and
den[g] = sum exp(s_n - SHIFT)  in one streaming pass and divide at the end
(the SHIFT cancels).

Sharding: 2048 graphs -> 8 cores x 8 groups x 32 graphs. Node rows of each
group are host-packed contiguously and padded to a common capacity C so all
cores run one identical NEFF. Per 128-node tile the device:
  - computes h^T = tanh(W1^T x^T + b1); x^T comes from a host-transposed
    fp8(e3m4) copy for nl tiles per chunk (PE upconverts in the matmul; the
    score path tolerates e3m4 on x, rel err ~6e-3) and from on-chip PE
    transposes of the fp16 x for the rest (hybrid balances PE time against
    HBM bytes; fp8 halves the transposed-copy bytes),
  - scores s = h^T.T @ W2 as a [128,1] column per tile, ex = exp(s+b2-SHIFT),
  - builds A[n, j] = ex_n * (iota_j == batch_rel_n) with two chunk-wide
    broadcast-AP tensor_tensor ops,
  - accumulates pooled^T += A.T @ x_aug into a PSUM bank. All pool matmuls
    of a chunk are emitted as one dense block (3-way PE column-tiling makes
    consecutive pool matmuls run ~3x concurrent; interleaving them with
    full-array matmuls would forfeit that concurrency). x_aug's ones column
    yields the denominators; per group the accumulator is reduced/divided/
    stored at group end.
The emission order software-pipelines chunk i's score phase against chunk
i-1's A-build (DVE) and pool block (PE col-masked) so every engine stays fed.
Group-end output DMAs ride the scalar queue so they never head-of-line-block
the sync queue's xa chunk loads.
"""

import os
import sys

for _p in ("/opt/trn_rl_repo",):
    if _p not in sys.path:
        sys.path.insert(0, _p)

import numpy as np

# ---------------------------------------------------------------- geometry
N_NODES = 1048576
HID = 256
HID2 = 128
G_TOTAL = 2048
N_CORES = 8
GT = 32            # graphs per pooling group (PSUM partition dim of pooled)
NG = 8             # groups per core
SEGS_PER_CORE = NG * GT          # 256
XW = HID + 2       # x_aug row width: 256 features + 1.0 + 1 pad zero
ONES_COL = HID     # column index of the ones column
CHUNK_TILES = 44   # 128-node tiles per DMA chunk (2.9 MB @ fp16)
ST = 4             # tiles per score supertile (512 nodes)
TK = int(os.environ.get("KERNEL_TK", "16"))
TK_HI = int(os.environ.get("KERNEL_TK_HI", "20"))
                   # per chunk: last TK tiles transpose x on-chip instead of
                   # loading the fp8 transposed copy. Multiple of ST.

# ---------------------------------------------------------------- dtypes
USE_FP16 = os.environ.get("KERNEL_FP16", "1") == "1"
USE_XT8 = os.environ.get("KERNEL_XT8", "1") == "1"
SHIFT = 8.0 if USE_FP16 else 0.0

_nc_cache = {}


def _dts():
    import concourse.mybir as mybir
    return mybir.dt.float16 if USE_FP16 else mybir.dt.float32


def _np_dts():
    return np.float16 if USE_FP16 else np.float32


def _np_xt_dt():
    if USE_XT8:
        import ml_dtypes
        return ml_dtypes.float8_e3m4
    return _np_dts()


def chunk_schedule(ntpg, ngroups=NG):
    """Shared host/device chunk list: (g, c0, nt, tk, nl, xt_off).

    The first chunk of group 0 and the last chunk of the last group are
    split so the pipeline fills fast (a small all-transposed chunk needs
    only its xa block) and drains fast (the final small chunk is fully
    loaded: no PE transposes on the critical drain path).
    """
    raw = [[(c0, min(CHUNK_TILES, ntpg - c0))
            for c0 in range(0, ntpg, CHUNK_TILES)] for g in range(ngroups)]
    c0, nt = raw[0][0]
    if nt >= 24:
        raw[0] = [(0, 12), (12, nt - 12)] + raw[0][1:]
    c0, nt = raw[-1][-1]
    if nt >= 24:
        raw[-1] = raw[-1][:-1] + [(c0, nt - 12), (c0 + nt - 12, 12)]
    flat = [(g, c0, nt) for g in range(ngroups) for c0, nt in raw[g]]
    out = []
    xt_off = 0
    for i, (g, c0, nt) in enumerate(flat):
        if g == ngroups - 1:
            nl = nt     # drain: the whole last group loads x^T from HBM, so
                        # its scores never wait on late xa and the tail after
                        # the final xa byte is just A-build+pool+finalize
        else:
            tk_want = TK_HI if g in (3, 4, 5) else TK
            # nl multiple of ST so no supertile mixes loaded+transposed
            nl = (max(0, nt - tk_want) // ST) * ST
        tk = nt - nl
        out.append((g, c0, nt, tk, nl, xt_off))
        xt_off += nl * 256
    return out


# ================================================================ device IR
def build_bass(ntpg, ngroups=NG, gt=GT, use_fp16=None):
    """Build + compile the per-core Bass program.

    ntpg: 128-node tiles per group (group capacity C = ntpg*128), mult of 4.
    """
    import concourse.bacc as bacc
    import concourse.mybir as mybir
    import concourse.tile as tile

    if use_fp16 is None:
        use_fp16 = USE_FP16
    dts = mybir.dt.float16 if use_fp16 else mybir.dt.float32
    dtx = mybir.dt.float8e3 if USE_XT8 else dts
    f32 = mybir.dt.float32
    AF = mybir.ActivationFunctionType
    OP = mybir.AluOpType

    assert ntpg % 2 == 0
    T = ngroups * ntpg                  # tiles per core
    S = T * 128                         # padded nodes per core

    nc = bacc.Bacc("TRN2", num_devices=N_CORES)

    sched = chunk_schedule(ntpg, ngroups)
    xtw = sched[-1][5] + sched[-1][4] * 256
    max_nt = max(s[2] for s in sched)

    # xa is host-swizzled partition-major: xa[p, t*XW + d] = x_aug[t*128 + p, d]
    # so any chunk of tiles is a contiguous 2D slice (big DMA runs).
    # xt is packed per chunk: [x^T chunk0 rows | x^T chunk1 rows] per chunk.
    xa = nc.dram_tensor("xa", [128, T * XW], dts, kind="ExternalInput").ap()
    xt = nc.dram_tensor("xt", [128, max(xtw, 256)], dtx,
                        kind="ExternalInput").ap()
    crel = nc.dram_tensor("crel", [128, T], dts, kind="ExternalInput").ap()
    w1 = nc.dram_tensor("w1", [HID, HID2], dts, kind="ExternalInput").ap()
    w2 = nc.dram_tensor("w2", [HID2, 1], dts, kind="ExternalInput").ap()
    b1c = nc.dram_tensor("b1c", [HID2, 1], f32, kind="ExternalInput").ap()
    b2c = nc.dram_tensor("b2c", [128, 1], f32, kind="ExternalInput").ap()
    iota = nc.dram_tensor("iota", [128, max_nt * gt], dts,
                          kind="ExternalInput").ap()
    blkid = nc.dram_tensor("blkid", [128, gt], f32, kind="ExternalInput").ap()
    ident = nc.dram_tensor("ident", [128, 128], dts, kind="ExternalInput").ap()
    out = nc.dram_tensor("out", [ngroups * gt, HID], f32, kind="ExternalOutput").ap()
    n_cg = 3                            # concurrent PE column-groups (PE
                                        # quadrant 3 is buggy; use 0..2)

    with tile.TileContext(nc) as tc:
        with (
            tc.tile_pool(name="consts", bufs=1) as cpool,
            tc.tile_pool(name="xa", bufs=4) as xa_pool,
            tc.tile_pool(name="xt", bufs=3) as xt_pool,
            tc.tile_pool(name="th", bufs=3) as th_pool,
            tc.tile_pool(name="ex", bufs=4) as ex_pool,
            tc.tile_pool(name="amat", bufs=2) as a_pool,
            tc.tile_pool(name="fin", bufs=2) as fin_pool,
            tc.tile_pool(name="xts", bufs=4) as xts_pool,
            tc.tile_pool(name="hp", bufs=2, space="PSUM") as hp_pool,
            tc.tile_pool(name="sp", bufs=1, space="PSUM") as sp_pool,
            tc.tile_pool(name="pp", bufs=2, space="PSUM") as pp_pool,
            tc.tile_pool(name="rp", bufs=1, space="PSUM") as rp_pool,
            tc.tile_pool(name="xtp", bufs=2, space="PSUM") as xtp_pool,
        ):
            # ---- small consts needed first (w1/ident gate the first matmuls)
            w1_sb = cpool.tile([128, 2 * HID2], dts)
            nc.sync.dma_start(out=w1_sb[:, 0:HID2], in_=w1[0:128, :])
            nc.sync.dma_start(out=w1_sb[:, HID2:2 * HID2], in_=w1[128:256, :])
            w2_sb = cpool.tile([128, 1], dts)
            nc.sync.dma_start(out=w2_sb[:], in_=w2[:])
            b1_sb = cpool.tile([128, 1], f32)
            nc.sync.dma_start(out=b1_sb[:], in_=b1c[:])
            b2_sb = cpool.tile([128, 1], f32)
            nc.sync.dma_start(out=b2_sb[:], in_=b2c[:])
            ident_sb = cpool.tile([128, 128], dts)
            nc.sync.dma_start(out=ident_sb[:], in_=ident[:])
            zeros_sb = cpool.tile([128, 512], f32)
            nc.gpsimd.memset(zeros_sb[:], 0.0)

            # larger consts ride behind chunk 0's data (they are only
            # needed by the A-build / group finalize, chunks later)
            iota_sb = cpool.tile([128, max_nt * gt], dts)
            blkid_sb = cpool.tile([128, gt], f32)
            crel_sb = cpool.tile([128, T], dts)

            def emit_late_consts():
                nc.sync.dma_start(out=iota_sb[:], in_=iota[:])
                nc.sync.dma_start(out=blkid_sb[:], in_=blkid[:])
                nc.sync.dma_start(out=crel_sb[:], in_=crel[:])

            # PE warmup: ~4us of dense matmuls so the HAM clock-gate opens
            # before the first real work (and while the first DMAs land).
            wu_ps = rp_pool.tile([128, 512], f32, space="PSUM", tag="red",
                                 name="wu")

            def emit_warmup():
                for _ in range(20):
                    nc.tensor.matmul(out=wu_ps[:], lhsT=zeros_sb[:, 0:128],
                                     rhs=zeros_sb[:], start=True, stop=True,
                                     skip_group_check=True)

            # software pipeline: chunk i's score phase is emitted interleaved
            # with chunk i-1's A-build (DVE), then chunk i-1's pool matmuls
            # follow as one dense col-tiled block.
            chunks = [(g, c0, nt) for g, c0, nt, tk, nl, xo in sched]

            state = {}       # chunk idx -> dict with tiles needed by pool
            group_ps = {}    # group -> pool accumulator

            def emit_dmas(i):
                g, c0, nt, tk, nl, xo = sched[i]
                t0_abs = g * ntpg + c0
                xa_sb = xa_pool.tile([128, nt * XW], dts, tag="xa")
                nc.sync.dma_start(
                    out=xa_sb[:], in_=xa[:, t0_abs * XW:(t0_abs + nt) * XW])
                st_ = {"xa": xa_sb, "nl": nl, "t0_abs": t0_abs, "g": g,
                       "c0": c0, "nt": nt}
                if nl:
                    xtb_sb = xt_pool.tile([128, nl * 256], dtx, tag="xtb")
                    nc.gpsimd.dma_start(
                        out=xtb_sb[:], in_=xt[:, xo:xo + nl * 256])
                    st_["xt0"] = xtb_sb[:, 0:nl * 128]
                    st_["xt1"] = xtb_sb[:, nl * 128:nl * 256]
                st_["sp"] = sp_pool.tile([128, nt], f32, space="PSUM", tag="sp",
                                         name="sp")
                state[i] = st_

            def score_ops(i):
                g, c0, nt = chunks[i]
                st_ = state[i]
                xa_sb, nl, sp = st_.get("xa"), st_["nl"], st_["sp"]

                def one_supertile(st):
                    k = min(ST, nt - st * ST)   # partial tail supertile ok
                    w = k * 128
                    hp = hp_pool.tile([128, w], f32, space="PSUM", tag="hp")
                    if st * ST >= nl:
                        # one PSUM bank holds the supertile's 2k trans-
                        # posed [128,128] blocks, laid out half-major so the
                        # h matmuls read two contiguous N=w slices
                        xtp = xtp_pool.tile([128, 2 * w], dts,
                                            space="PSUM", tag="xtp")
                        for pr in range(k // 2):
                            t_lo = st * ST + pr * 2
                            for u in range(2):
                                for c in range(2):
                                    o = c * w + (pr * 2 + u) * 128
                                    nc.tensor.transpose(
                                        out=xtp[:, o:o + 128],
                                        in_=xa_sb[:, (t_lo + u) * XW + c * 128:
                                                  (t_lo + u) * XW + (c + 1) * 128],
                                        identity=ident_sb[:])
                        xts = xts_pool.tile([128, 2 * w], dts, tag="xts")
                        # PSUM->SBUF copies split DVE:ACT ~5:1 (DVE copy
                        # ~850ns, ACT copy ~1200ns; ACT is tanh-heavy)
                        if (st - (nl // ST)) % 6 < 5:
                            nc.vector.tensor_copy(xts[:], xtp[:])
                        else:
                            nc.scalar.copy(xts[:], xtp[:])
                        rhs0, rhs1 = xts[:, 0:w], xts[:, w:2 * w]
                    else:
                        rhs0 = st_["xt0"][:, st * ST * 128:st * ST * 128 + w]
                        rhs1 = st_["xt1"][:, st * ST * 128:st * ST * 128 + w]
                    # w2 scores lag one supertile (tanh long done) and sit
                    # between the transposes and the copy-dependent h matmuls
                    # so the in-order PE queue has ready work during the
                    # PSUM->SBUF copy
                    if st > 0:
                        w2_block(st - 1)
                    nc.tensor.matmul(
                        out=hp[:], lhsT=w1_sb[:, 0:HID2],
                        rhs=rhs0, start=True, stop=False)
                    nc.tensor.matmul(
                        out=hp[:], lhsT=w1_sb[:, HID2:2 * HID2],
                        rhs=rhs1, start=False, stop=True)
                    th = th_pool.tile([128, w], dts, tag="th")
                    nc.scalar.activation(th[:], hp[:], AF.Tanh,
                                         bias=b1_sb[:, 0:1])
                    st_.setdefault("th", {})[st] = th

                def w2_block(st):
                    th = st_["th"].pop(st)
                    for j in range(min(ST, nt - st * ST)):
                        jj = st * ST + j
                        nc.tensor.matmul(
                            out=sp[:, jj:jj + 1],
                            lhsT=th[:, j * 128:(j + 1) * 128],
                            rhs=w2_sb[:],
                            start=(jj == 0), stop=(jj == nt - 1),
                            skip_group_check=True)

                def fin():
                    w2_block((nt + ST - 1) // ST - 1)
                    ex = ex_pool.tile([128, nt], dts, tag="ex")
                    nc.scalar.activation(ex[:], sp[:], AF.Exp,
                                         bias=b2_sb[:, 0:1])
                    st_["ex"] = ex

                return [lambda st=st: one_supertile(st)
                        for st in range((nt + ST - 1) // ST)] + [fin]

            def a4_ops(i):
                """Chunk-wide A-matrix build: 2 DVE ops over [128, nt*gt]."""
                g, c0, nt = chunks[i]
                st_ = state[i]
                t0_abs = st_["t0_abs"]

                def build_eq():
                    a4 = a_pool.tile([128, nt * gt], dts, tag="a4")
                    st_["a4"] = a4
                    nc.vector.tensor_tensor(
                        out=a4[:].rearrange("p (t o) -> p t o", o=gt),
                        in0=iota_sb[:, 0:nt * gt].rearrange(
                            "p (t o) -> p t o", o=gt),
                        in1=crel_sb[:, t0_abs:t0_abs + nt].broadcast_to(
                            [128, nt, gt]),
                        op=OP.is_equal)

                def build_mul():
                    a4 = st_["a4"]
                    a4v = a4[:].rearrange("p (t o) -> p t o", o=gt)
                    nc.vector.tensor_tensor(
                        out=a4v, in0=a4v,
                        in1=st_["ex"][:].broadcast_to([128, nt, gt]),
                        op=OP.mult)

                return [build_eq, build_mul]

            def pool_block(i):
                """Dense col-tiled pool matmul block for chunk i."""
                g, c0, nt = chunks[i]
                st_ = state[i]
                xa_sb = st_["xa"]

                def run():
                    if c0 == 0:
                        pool_ps = pp_pool.tile([128, 512], f32, space="PSUM",
                                               tag="pool")
                        group_ps[g] = pool_ps
                        nc.tensor.matmul(
                            out=pool_ps[:], lhsT=zeros_sb[:, 0:128],
                            rhs=zeros_sb[:],
                            start=True, stop=False, skip_group_check=True)
                    pool_ps = group_ps[g]
                    a4 = st_["a4"]
                    for j in range(nt):
                        t_in_g = c0 + j
                        a = t_in_g % n_cg
                        nc.tensor.matmul(
                            out=pool_ps[gt * a:gt * (a + 1), 0:XW],
                            lhsT=a4[:, j * gt:(j + 1) * gt],
                            rhs=xa_sb[:, j * XW:(j + 1) * XW],
                            start=False, stop=(t_in_g == ntpg - 1),
                            tile_position=(0, gt * a),
                            skip_group_check=True)
                    if c0 + nt >= ntpg:
                        pool_ps = group_ps.pop(g)
                        acc_sb = fin_pool.tile([128, XW], f32, tag="acc")
                        nc.vector.tensor_copy(acc_sb[:], pool_ps[:, 0:XW])
                        red_ps = rp_pool.tile([gt, XW], f32, space="PSUM",
                                              tag="red")
                        nc.tensor.matmul(out=red_ps[:], lhsT=blkid_sb[:],
                                         rhs=acc_sb[:], start=True, stop=True)
                        rec = fin_pool.tile([gt, 1], f32, tag="rec")
                        nc.vector.reciprocal(
                            rec[:], red_ps[:, ONES_COL:ONES_COL + 1])
                        og = fin_pool.tile([gt, HID], f32, tag="og")
                        nc.vector.tensor_scalar(
                            og[:], red_ps[:, 0:HID], rec[:, 0:1], None, OP.mult)
                        # scalar queue: never head-of-line-blocks xa loads
                        nc.scalar.dma_start(out=out[g * gt:(g + 1) * gt, :],
                                            in_=og[:])
                    del state[i]

                return run

            emit_dmas(0)
            emit_warmup()
            emit_late_consts()
            for i in range(len(chunks) + 1):
                s_ops = score_ops(i) if i < len(chunks) else []
                a_ops = a4_ops(i - 1) if i > 0 else []
                p_run = pool_block(i - 1) if i > 0 else None
                k = max(len(s_ops), len(a_ops))
                for q in range(k):
                    if q < len(s_ops):
                        s_ops[q]()
                    if q == 0 and i + 1 < len(chunks):
                        emit_dmas(i + 1)
                    if q < len(a_ops):
                        a_ops[q]()
                if p_run is not None:
                    p_run()

    nc.compile()
    return nc


# ================================================================ host prep
def pack_groups(counts, n_bins, gt):
    """Greedy bin-packing: graphs -> bins of exactly gt graphs, balancing
    node load so the padded group capacity C shrinks. Returns
    (graphs_of_bin[b] lists, slot_of_graph)."""
    import heapq
    order = np.argsort(-counts, kind="stable")
    load = np.zeros(n_bins, np.int64)
    members = [[] for _ in range(n_bins)]
    h = [(0, b) for b in range(n_bins)]
    heapq.heapify(h)
    for gid in order:
        popped = []
        while True:
            l, b = heapq.heappop(h)
            if len(members[b]) < gt:
                break
            popped.append((l, b))
        for p in popped:
            heapq.heappush(h, p)
        members[b].append(int(gid))
        load[b] += int(counts[gid])
        if len(members[b]) < gt:
            heapq.heappush(h, (int(load[b]), b))
    slot = np.zeros(len(counts), np.int64)
    for b in range(n_bins):
        for s, gid in enumerate(members[b]):
            slot[gid] = s
    return members, slot, int(load.max())


def prepare_shards(x, batch, W1, b1, W2, b2, ngroups=NG, gt=GT, n_cores=N_CORES):
    """Split nodes into (core, group) node blocks padded to capacity C."""
    np_dts = _np_dts()
    np_xt = _np_xt_dt()
    x = np.asarray(x)
    batch = np.asarray(batch).astype(np.int64)
    g_total = n_cores * ngroups * gt
    counts = np.bincount(batch, minlength=g_total)
    starts = np.concatenate([[0], np.cumsum(counts)])[:-1]
    n_bins = n_cores * ngroups
    members, slot, max_load = pack_groups(counts, n_bins, gt)
    # C granularity is 2 tiles (ST even): partial supertiles handle the tail
    C = int(max(512, ((max_load + 255) // 256) * 256))
    ntpg = C // 128
    T = ngroups * ntpg

    sched = chunk_schedule(ntpg, ngroups)
    max_nt = max(s[2] for s in sched)

    w1c = np.ascontiguousarray(W1).astype(np_dts)
    w2c = np.ascontiguousarray(W2).astype(np_dts)
    b1c = np.asarray(b1, np.float32).reshape(HID2, 1)
    b2c = np.full((128, 1), float(np.asarray(b2).reshape(-1)[0]) - SHIFT,
                  np.float32)
    iota = np.tile(np.arange(gt, dtype=np.float32), (128, max_nt)).astype(np_dts)
    blkid = np.zeros((128, gt), np.float32)
    blkid[np.arange(128), np.arange(128) % gt] = 1.0

    # out row for graph gid: core*ngroups*gt + group*gt + slot
    pos = np.zeros(g_total, np.int64)
    in_maps = []
    for core in range(n_cores):
        xa = np.zeros((ngroups * C, XW), np.float32)
        crel_flat = np.full(ngroups * C, -1.0, np.float32)
        for g in range(ngroups):
            b = core * ngroups + g
            off = g * C
            for s, gid in enumerate(members[b]):
                s0, n = int(starts[gid]), int(counts[gid])
                xa[off:off + n, :HID] = x[s0:s0 + n]
                crel_flat[off:off + n] = float(s)
                pos[gid] = (core * ngroups + g) * gt + s
                off += n
        xa[:, ONES_COL] = 1.0
        xtfull = np.ascontiguousarray(xa[:, :HID].T).astype(np_xt)
        # pack x^T per chunk (both hidden halves back to back, contiguous)
        xtw = sched[-1][5] + sched[-1][4] * 256
        xt = np.zeros((128, max(xtw, 256)), np_xt)
        for g, c0, nt, tk, nl, xo in sched:
            n0 = (g * ntpg + c0) * 128
            xt[:, xo:xo + nl * 128] = xtfull[0:128, n0:n0 + nl * 128]
            xt[:, xo + nl * 128:xo + nl * 256] = xtfull[128:256, n0:n0 + nl * 128]
        # partition-major swizzle: xa_swz[p, t*XW + d] = xa[t*128 + p, d]
        xa_swz = np.ascontiguousarray(
            xa.astype(np_dts).reshape(T, 128, XW).transpose(1, 0, 2)
        ).reshape(128, T * XW)
        in_maps.append({
            "xa": xa_swz,
            "xt": xt,
            "crel": np.ascontiguousarray(crel_flat.reshape(T, 128).T)
                      .astype(np_dts),
            "w1": w1c, "w2": w2c, "b1c": b1c, "b2c": b2c, "iota": iota,
            "blkid": blkid, "ident": np.eye(128, dtype=np_dts),
        })
    return in_maps, ntpg, pos


# ================================================================ entry
LAST_RESULTS = None


def kernel(x, batch, W1, b1, W2, b2):
    global LAST_RESULTS
    from concourse.bass_utils import run_bass_kernel_spmd

    in_maps, ntpg, pos = prepare_shards(x, batch, W1, b1, W2, b2)
    key = (ntpg, USE_FP16, USE_XT8, TK)
    if key not in _nc_cache:
        _nc_cache[key] = build_bass(ntpg)
    nc = _nc_cache[key]
    trace = os.environ.get("KERNEL_TRACE", "0") == "1"
    res = run_bass_kernel_spmd(nc, in_maps, core_ids=list(range(N_CORES)),
                               trace=trace)
    LAST_RESULTS = res
    pooled = np.concatenate([r["out"] for r in res.results], axis=0)
    return pooled[pos].astype(np.float32)


# revision 13
# speedup vs baseline: 1.0255x; 1.0255x over previous
"""AttentivePooling Trainium2 kernel (8 NeuronCores, SPMD).

Math (per graph g):  pooled[g] = sum_{n in g} softmax_g(s)_n * x[n]
with s_n = tanh(x W1 + b1) W2 + b2.  Since tanh bounds |s| <= ||W2||_1 + |b2|
(~9 for these inputs), the segment-max subtraction in the reference is
unnecessary: we accumulate  num[g] = sum exp(s_n - SHIFT) x_n  and
den[g] = sum exp(s_n - SHIFT)  in one streaming pass and divide at the end
(the SHIFT cancels).

Sharding: 2048 graphs -> 8 cores x 8 groups x 32 graphs. Graphs are
greedy-bin-packed across the 64 (core,group) bins to balance node counts, so
the common padded capacity C (granularity 256 nodes; partial 2-tile
supertiles handle the tail) stays ~1.5% above the mean; the host undoes the
permutation after the gather. All cores run one identical NEFF. Per 128-node
tile the device:
  - computes h^T = tanh(W1^T x^T + b1); x^T comes from a host-transposed
    fp8(e3m4) copy for nl tiles per chunk (PE upconverts in the matmul; the
    score path tolerates e3m4 on x, rel err ~6e-3) and from on-chip PE
    transposes of the fp16 x for the rest (hybrid balances PE time against
    HBM bytes; fp8 halves the transposed-copy bytes),
  - scores s = h^T.T @ W2 as a [128,1] column per tile, ex = exp(s+b2-SHIFT),
  - builds A[n, j] = ex_n * (iota_j == batch_rel_n) with two chunk-wide
    broadcast-AP tensor_tensor ops,
  - accumulates pooled^T += A.T @ x_aug into a PSUM bank. All pool matmuls
    of a chunk are emitted as one dense block (3-way PE column-tiling makes
    consecutive pool matmuls run ~3x concurrent; interleaving them with
    full-array matmuls would forfeit that concurrency). x_aug's ones column
    yields the denominators; per group the accumulator is reduced/divided/
    stored at group end.
The emission order software-pipelines chunk i's score phase against chunk
i-1's A-build (DVE) and pool block (PE col-masked) so every engine stays fed.
Group-end output DMAs ride the scalar queue so they never head-of-line-block
the sync queue's xa chunk loads. The last group is fully loaded (no on-chip
transposes) so its scores depend only on the early fp8 x^T stream and the
post-last-DMA tail is just A-build + pool + finalize; groups 3-5 carry extra
transposes (TK_HI) to keep PE fed where the DMA stream is the long pole.

Measured on 8 axon trn2 cores: ~314-325us (device-thermal variance), vs
397.6us baseline; rel err ~5.7e-3 (gate 2e-2). Engine occupancy at best run:
PE 92% busy (the long pole), DMA ~85%, ACT ~60%, DVE ~47%.
"""

import os
import sys

for _p in ("/opt/trn_rl_repo",):
    if _p not in sys.path:
        sys.path.insert(0, _p)

import numpy as np

# ---------------------------------------------------------------- geometry
N_NODES = 1048576
HID = 256
HID2 = 128
G_TOTAL = 2048
N_CORES = 8
GT = 32            # graphs per pooling group (PSUM partition dim of pooled)
NG = 8             # groups per core
SEGS_PER_CORE = NG * GT          # 256
XW = HID + 2       # x_aug row width: 256 features + 1.0 + 1 pad zero
ONES_COL = HID     # column index of the ones column
CHUNK_TILES = 44   # 128-node tiles per DMA chunk (2.9 MB @ fp16)
ST = 4             # tiles per score supertile (512 nodes)
TK = int(os.environ.get("KERNEL_TK", "16"))
TK_HI = int(os.environ.get("KERNEL_TK_HI", "20"))
                   # per chunk: last TK tiles transpose x on-chip instead of
                   # loading the fp8 transposed copy. Multiple of ST.

# ---------------------------------------------------------------- dtypes
USE_FP16 = os.environ.get("KERNEL_FP16", "1") == "1"
USE_XT8 = os.environ.get("KERNEL_XT8", "1") == "1"
SHIFT = 8.0 if USE_FP16 else 0.0

_nc_cache = {}


def _dts():
    import concourse.mybir as mybir
    return mybir.dt.float16 if USE_FP16 else mybir.dt.float32


def _np_dts():
    return np.float16 if USE_FP16 else np.float32


def _np_xt_dt():
    if USE_XT8:
        import ml_dtypes
        return ml_dtypes.float8_e3m4
    return _np_dts()


def chunk_schedule(ntpg, ngroups=NG):
    """Shared host/device chunk list: (g, c0, nt, tk, nl, xt_off).

    The first chunk of group 0 and the last chunk of the last group are
    split so the pipeline fills fast (a small all-transposed chunk needs
    only its xa block) and drains fast (the final small chunk is fully
    loaded: no PE transposes on the critical drain path).
    """
    raw = [[(c0, min(CHUNK_TILES, ntpg - c0))
            for c0 in range(0, ntpg, CHUNK_TILES)] for g in range(ngroups)]
    c0, nt = raw[0][0]
    if nt >= 24:
        raw[0] = [(0, 12), (12, nt - 12)] + raw[0][1:]
    c0, nt = raw[-1][-1]
    if nt >= 24:
        raw[-1] = raw[-1][:-1] + [(c0, nt - 12), (c0 + nt - 12, 12)]
    flat = [(g, c0, nt) for g in range(ngroups) for c0, nt in raw[g]]
    out = []
    xt_off = 0
    for i, (g, c0, nt) in enumerate(flat):
        if g == ngroups - 1:
            nl = nt     # drain: the whole last group loads x^T from HBM, so
                        # its scores never wait on late xa and the tail after
                        # the final xa byte is just A-build+pool+finalize
        else:
            tk_want = TK_HI if g in (3, 4, 5) else TK
            # nl multiple of ST so no supertile mixes loaded+transposed
            nl = (max(0, nt - tk_want) // ST) * ST
        tk = nt - nl
        out.append((g, c0, nt, tk, nl, xt_off))
        xt_off += nl * 256
    return out


# ================================================================ device IR
def build_bass(ntpg, ngroups=NG, gt=GT, use_fp16=None):
    """Build + compile the per-core Bass program.

    ntpg: 128-node tiles per group (group capacity C = ntpg*128), mult of 4.
    """
    import concourse.bacc as bacc
    import concourse.mybir as mybir
    import concourse.tile as tile

    if use_fp16 is None:
        use_fp16 = USE_FP16
    dts = mybir.dt.float16 if use_fp16 else mybir.dt.float32
    dtx = mybir.dt.float8e3 if USE_XT8 else dts
    f32 = mybir.dt.float32
    AF = mybir.ActivationFunctionType
    OP = mybir.AluOpType

    assert ntpg % 2 == 0
    T = ngroups * ntpg                  # tiles per core
    S = T * 128                         # padded nodes per core

    nc = bacc.Bacc("TRN2", num_devices=N_CORES)

    sched = chunk_schedule(ntpg, ngroups)
    xtw = sched[-1][5] + sched[-1][4] * 256
    max_nt = max(s[2] for s in sched)

    # xa is host-swizzled partition-major: xa[p, t*XW + d] = x_aug[t*128 + p, d]
    # so any chunk of tiles is a contiguous 2D slice (big DMA runs).
    # xt is packed per chunk: [x^T chunk0 rows | x^T chunk1 rows] per chunk.
    xa = nc.dram_tensor("xa", [128, T * XW], dts, kind="ExternalInput").ap()
    xt = nc.dram_tensor("xt", [128, max(xtw, 256)], dtx,
                        kind="ExternalInput").ap()
    crel = nc.dram_tensor("crel", [128, T], dts, kind="ExternalInput").ap()
    w1 = nc.dram_tensor("w1", [HID, HID2], dts, kind="ExternalInput").ap()
    w2 = nc.dram_tensor("w2", [HID2, 1], dts, kind="ExternalInput").ap()
    b1c = nc.dram_tensor("b1c", [HID2, 1], f32, kind="ExternalInput").ap()
    b2c = nc.dram_tensor("b2c", [128, 1], f32, kind="ExternalInput").ap()
    iota = nc.dram_tensor("iota", [128, max_nt * gt], dts,
                          kind="ExternalInput").ap()
    blkid = nc.dram_tensor("blkid", [128, gt], f32, kind="ExternalInput").ap()
    ident = nc.dram_tensor("ident", [128, 128], dts, kind="ExternalInput").ap()
    out = nc.dram_tensor("out", [ngroups * gt, HID], f32, kind="ExternalOutput").ap()
    n_cg = 3                            # concurrent PE column-groups (PE
                                        # quadrant 3 is buggy; use 0..2)

    with tile.TileContext(nc) as tc:
        with (
            tc.tile_pool(name="consts", bufs=1) as cpool,
            tc.tile_pool(name="xa", bufs=4) as xa_pool,
            tc.tile_pool(name="xt", bufs=3) as xt_pool,
            tc.tile_pool(name="th", bufs=3) as th_pool,
            tc.tile_pool(name="ex", bufs=4) as ex_pool,
            tc.tile_pool(name="amat", bufs=2) as a_pool,
            tc.tile_pool(name="fin", bufs=2) as fin_pool,
            tc.tile_pool(name="xts", bufs=4) as xts_pool,
            tc.tile_pool(name="hp", bufs=2, space="PSUM") as hp_pool,
            tc.tile_pool(name="sp", bufs=1, space="PSUM") as sp_pool,
            tc.tile_pool(name="pp", bufs=2, space="PSUM") as pp_pool,
            tc.tile_pool(name="rp", bufs=1, space="PSUM") as rp_pool,
            tc.tile_pool(name="xtp", bufs=2, space="PSUM") as xtp_pool,
        ):
            # ---- small consts needed first (w1/ident gate the first matmuls)
            w1_sb = cpool.tile([128, 2 * HID2], dts)
            nc.sync.dma_start(out=w1_sb[:, 0:HID2], in_=w1[0:128, :])
            nc.sync.dma_start(out=w1_sb[:, HID2:2 * HID2], in_=w1[128:256, :])
            w2_sb = cpool.tile([128, 1], dts)
            nc.sync.dma_start(out=w2_sb[:], in_=w2[:])
            b1_sb = cpool.tile([128, 1], f32)
            nc.sync.dma_start(out=b1_sb[:], in_=b1c[:])
            b2_sb = cpool.tile([128, 1], f32)
            nc.sync.dma_start(out=b2_sb[:], in_=b2c[:])
            ident_sb = cpool.tile([128, 128], dts)
            nc.sync.dma_start(out=ident_sb[:], in_=ident[:])
            zeros_sb = cpool.tile([128, 512], f32)
            nc.gpsimd.memset(zeros_sb[:], 0.0)

            # larger consts ride behind chunk 0's data (they are only
            # needed by the A-build / group finalize, chunks later)
            iota_sb = cpool.tile([128, max_nt * gt], dts)
            blkid_sb = cpool.tile([128, gt], f32)
            crel_sb = cpool.tile([128, T], dts)

            def emit_late_consts():
                nc.sync.dma_start(out=iota_sb[:], in_=iota[:])
                nc.sync.dma_start(out=blkid_sb[:], in_=blkid[:])
                nc.sync.dma_start(out=crel_sb[:], in_=crel[:])

            # PE warmup: ~4us of dense matmuls so the HAM clock-gate opens
            # before the first real work (and while the first DMAs land).
            wu_ps = rp_pool.tile([128, 512], f32, space="PSUM", tag="red",
                                 name="wu")

            def emit_warmup():
                for _ in range(20):
                    nc.tensor.matmul(out=wu_ps[:], lhsT=zeros_sb[:, 0:128],
                                     rhs=zeros_sb[:], start=True, stop=True,
                                     skip_group_check=True)

            # software pipeline: chunk i's score phase is emitted interleaved
            # with chunk i-1's A-build (DVE), then chunk i-1's pool matmuls
            # follow as one dense col-tiled block.
            chunks = [(g, c0, nt) for g, c0, nt, tk, nl, xo in sched]

            state = {}       # chunk idx -> dict with tiles needed by pool
            group_ps = {}    # group -> pool accumulator

            def emit_dmas(i):
                g, c0, nt, tk, nl, xo = sched[i]
                t0_abs = g * ntpg + c0
                xa_sb = xa_pool.tile([128, nt * XW], dts, tag="xa")
                nc.sync.dma_start(
                    out=xa_sb[:], in_=xa[:, t0_abs * XW:(t0_abs + nt) * XW])
                st_ = {"xa": xa_sb, "nl": nl, "t0_abs": t0_abs, "g": g,
                       "c0": c0, "nt": nt}
                if nl:
                    xtb_sb = xt_pool.tile([128, nl * 256], dtx, tag="xtb")
                    nc.gpsimd.dma_start(
                        out=xtb_sb[:], in_=xt[:, xo:xo + nl * 256])
                    st_["xt0"] = xtb_sb[:, 0:nl * 128]
                    st_["xt1"] = xtb_sb[:, nl * 128:nl * 256]
                st_["sp"] = sp_pool.tile([128, nt], f32, space="PSUM", tag="sp",
                                         name="sp")
                state[i] = st_

            def score_ops(i):
                g, c0, nt = chunks[i]
                st_ = state[i]
                xa_sb, nl, sp = st_.get("xa"), st_["nl"], st_["sp"]

                def one_supertile(st):
                    k = min(ST, nt - st * ST)   # partial tail supertile ok
                    w = k * 128
                    hp = hp_pool.tile([128, w], f32, space="PSUM", tag="hp")
                    if st * ST >= nl:
                        # one PSUM bank holds the supertile's 2k trans-
                        # posed [128,128] blocks, laid out half-major so the
                        # h matmuls read two contiguous N=w slices
                        xtp = xtp_pool.tile([128, 2 * w], dts,
                                            space="PSUM", tag="xtp")
                        for pr in range(k // 2):
                            t_lo = st * ST + pr * 2
                            for u in range(2):
                                for c in range(2):
                                    o = c * w + (pr * 2 + u) * 128
                                    nc.tensor.transpose(
                                        out=xtp[:, o:o + 128],
                                        in_=xa_sb[:, (t_lo + u) * XW + c * 128:
                                                  (t_lo + u) * XW + (c + 1) * 128],
                                        identity=ident_sb[:])
                        xts = xts_pool.tile([128, 2 * w], dts, tag="xts")
                        # PSUM->SBUF copies split DVE:ACT ~5:1 (DVE copy
                        # ~850ns, ACT copy ~1200ns; ACT is tanh-heavy)
                        if (st - (nl // ST)) % 6 < 5:
                            nc.vector.tensor_copy(xts[:], xtp[:])
                        else:
                            nc.scalar.copy(xts[:], xtp[:])
                        rhs0, rhs1 = xts[:, 0:w], xts[:, w:2 * w]
                    else:
                        rhs0 = st_["xt0"][:, st * ST * 128:st * ST * 128 + w]
                        rhs1 = st_["xt1"][:, st * ST * 128:st * ST * 128 + w]
                    # w2 scores lag one supertile (tanh long done) and sit
                    # between the transposes and the copy-dependent h matmuls
                    # so the in-order PE queue has ready work during the
                    # PSUM->SBUF copy
                    if st > 0:
                        w2_block(st - 1)
                    nc.tensor.matmul(
                        out=hp[:], lhsT=w1_sb[:, 0:HID2],
                        rhs=rhs0, start=True, stop=False)
                    nc.tensor.matmul(
                        out=hp[:], lhsT=w1_sb[:, HID2:2 * HID2],
                        rhs=rhs1, start=False, stop=True)
                    th = th_pool.tile([128, w], dts, tag="th")
                    nc.scalar.activation(th[:], hp[:], AF.Tanh,
                                         bias=b1_sb[:, 0:1])
                    st_.setdefault("th", {})[st] = th

                def w2_block(st):
                    th = st_["th"].pop(st)
                    for j in range(min(ST, nt - st * ST)):
                        jj = st * ST + j
                        nc.tensor.matmul(
                            out=sp[:, jj:jj + 1],
                            lhsT=th[:, j * 128:(j + 1) * 128],
                            rhs=w2_sb[:],
                            start=(jj == 0), stop=(jj == nt - 1),
                            skip_group_check=True)

                def fin():
                    w2_block((nt + ST - 1) // ST - 1)
                    ex = ex_pool.tile([128, nt], dts, tag="ex")
                    nc.scalar.activation(ex[:], sp[:], AF.Exp,
                                         bias=b2_sb[:, 0:1])
                    st_["ex"] = ex

                return [lambda st=st: one_supertile(st)
                        for st in range((nt + ST - 1) // ST)] + [fin]

            def a4_ops(i):
                """Chunk-wide A-matrix build: 2 DVE ops over [128, nt*gt]."""
                g, c0, nt = chunks[i]
                st_ = state[i]
                t0_abs = st_["t0_abs"]

                def build_eq():
                    a4 = a_pool.tile([128, nt * gt], dts, tag="a4")
                    st_["a4"] = a4
                    nc.vector.tensor_tensor(
                        out=a4[:].rearrange("p (t o) -> p t o", o=gt),
                        in0=iota_sb[:, 0:nt * gt].rearrange(
                            "p (t o) -> p t o", o=gt),
                        in1=crel_sb[:, t0_abs:t0_abs + nt].broadcast_to(
                            [128, nt, gt]),
                        op=OP.is_equal)

                def build_mul():
                    a4 = st_["a4"]
                    a4v = a4[:].rearrange("p (t o) -> p t o", o=gt)
                    nc.vector.tensor_tensor(
                        out=a4v, in0=a4v,
                        in1=st_["ex"][:].broadcast_to([128, nt, gt]),
                        op=OP.mult)

                return [build_eq, build_mul]

            def pool_block(i):
                """Dense col-tiled pool matmul block for chunk i."""
                g, c0, nt = chunks[i]
                st_ = state[i]
                xa_sb = st_["xa"]

                def run():
                    if c0 == 0:
                        pool_ps = pp_pool.tile([128, 512], f32, space="PSUM",
                                               tag="pool")
                        group_ps[g] = pool_ps
                        nc.tensor.matmul(
                            out=pool_ps[:], lhsT=zeros_sb[:, 0:128],
                            rhs=zeros_sb[:],
                            start=True, stop=False, skip_group_check=True)
                    pool_ps = group_ps[g]
                    a4 = st_["a4"]
                    for j in range(nt):
                        t_in_g = c0 + j
                        a = t_in_g % n_cg
                        nc.tensor.matmul(
                            out=pool_ps[gt * a:gt * (a + 1), 0:XW],
                            lhsT=a4[:, j * gt:(j + 1) * gt],
                            rhs=xa_sb[:, j * XW:(j + 1) * XW],
                            start=False, stop=(t_in_g == ntpg - 1),
                            tile_position=(0, gt * a),
                            skip_group_check=True)
                    if c0 + nt >= ntpg:
                        pool_ps = group_ps.pop(g)
                        acc_sb = fin_pool.tile([128, XW], f32, tag="acc")
                        nc.vector.tensor_copy(acc_sb[:], pool_ps[:, 0:XW])
                        red_ps = rp_pool.tile([gt, XW], f32, space="PSUM",
                                              tag="red")
                        nc.tensor.matmul(out=red_ps[:], lhsT=blkid_sb[:],
                                         rhs=acc_sb[:], start=True, stop=True)
                        rec = fin_pool.tile([gt, 1], f32, tag="rec")
                        nc.vector.reciprocal(
                            rec[:], red_ps[:, ONES_COL:ONES_COL + 1])
                        og = fin_pool.tile([gt, HID], f32, tag="og")
                        nc.vector.tensor_scalar(
                            og[:], red_ps[:, 0:HID], rec[:, 0:1], None, OP.mult)
                        # scalar queue: never head-of-line-blocks xa loads
                        nc.scalar.dma_start(out=out[g * gt:(g + 1) * gt, :],
                                            in_=og[:])
                    del state[i]

                return run

            emit_dmas(0)
            emit_warmup()
            emit_late_consts()
            for i in range(len(chunks) + 1):
                s_ops = score_ops(i) if i < len(chunks) else []
                a_ops = a4_ops(i - 1) if i > 0 else []
                p_run = pool_block(i - 1) if i > 0 else None
                k = max(len(s_ops), len(a_ops))
                for q in range(k):
                    if q < len(s_ops):
                        s_ops[q]()
                    if q == 0 and i + 1 < len(chunks):
                        emit_dmas(i + 1)
                    if q < len(a_ops):
                        a_ops[q]()
                if p_run is not None:
                    p_run()

    nc.compile()
    return nc


# ================================================================ host prep
def pack_groups(counts, n_bins, gt):
    """Greedy bin-packing: graphs -> bins of exactly gt graphs, balancing
    node load so the padded group capacity C shrinks. Returns
    (graphs_of_bin[b] lists, slot_of_graph)."""
    import heapq
    order = np.argsort(-counts, kind="stable")
    load = np.zeros(n_bins, np.int64)
    members = [[] for _ in range(n_bins)]
    h = [(0, b) for b in range(n_bins)]
    heapq.heapify(h)
    for gid in order:
        popped = []
        while True:
            l, b = heapq.heappop(h)
            if len(members[b]) < gt:
                break
            popped.append((l, b))
        for p in popped:
            heapq.heappush(h, p)
        members[b].append(int(gid))
        load[b] += int(counts[gid])
        if len(members[b]) < gt:
            heapq.heappush(h, (int(load[b]), b))
    slot = np.zeros(len(counts), np.int64)
    for b in range(n_bins):
        for s, gid in enumerate(members[b]):
            slot[gid] = s
    return members, slot, int(load.max())


def prepare_shards(x, batch, W1, b1, W2, b2, ngroups=NG, gt=GT, n_cores=N_CORES):
    """Split nodes into (core, group) node blocks padded to capacity C."""
    np_dts = _np_dts()
    np_xt = _np_xt_dt()
    x = np.asarray(x)
    batch = np.asarray(batch).astype(np.int64)
    g_total = n_cores * ngroups * gt
    counts = np.bincount(batch, minlength=g_total)
    starts = np.concatenate([[0], np.cumsum(counts)])[:-1]
    n_bins = n_cores * ngroups
    members, slot, max_load = pack_groups(counts, n_bins, gt)
    # C granularity is 2 tiles (ST even): partial supertiles handle the tail
    C = int(max(512, ((max_load + 255) // 256) * 256))
    ntpg = C // 128
    T = ngroups * ntpg

    sched = chunk_schedule(ntpg, ngroups)
    max_nt = max(s[2] for s in sched)

    w1c = np.ascontiguousarray(W1).astype(np_dts)
    w2c = np.ascontiguousarray(W2).astype(np_dts)
    b1c = np.asarray(b1, np.float32).reshape(HID2, 1)
    b2c = np.full((128, 1), float(np.asarray(b2).reshape(-1)[0]) - SHIFT,
                  np.float32)
    iota = np.tile(np.arange(gt, dtype=np.float32), (128, max_nt)).astype(np_dts)
    blkid = np.zeros((128, gt), np.float32)
    blkid[np.arange(128), np.arange(128) % gt] = 1.0

    # out row for graph gid: core*ngroups*gt + group*gt + slot
    pos = np.zeros(g_total, np.int64)
    in_maps = []
    for core in range(n_cores):
        xa = np.zeros((ngroups * C, XW), np.float32)
        crel_flat = np.full(ngroups * C, -1.0, np.float32)
        for g in range(ngroups):
            b = core * ngroups + g
            off = g * C
            for s, gid in enumerate(members[b]):
                s0, n = int(starts[gid]), int(counts[gid])
                xa[off:off + n, :HID] = x[s0:s0 + n]
                crel_flat[off:off + n] = float(s)
                pos[gid] = (core * ngroups + g) * gt + s
                off += n
        xa[:, ONES_COL] = 1.0
        xtfull = np.ascontiguousarray(xa[:, :HID].T).astype(np_xt)
        # pack x^T per chunk (both hidden halves back to back, contiguous)
        xtw = sched[-1][5] + sched[-1][4] * 256
        xt = np.zeros((128, max(xtw, 256)), np_xt)
        for g, c0, nt, tk, nl, xo in sched:
            n0 = (g * ntpg + c0) * 128
            xt[:, xo:xo + nl * 128] = xtfull[0:128, n0:n0 + nl * 128]
            xt[:, xo + nl * 128:xo + nl * 256] = xtfull[128:256, n0:n0 + nl * 128]
        # partition-major swizzle: xa_swz[p, t*XW + d] = xa[t*128 + p, d]
        xa_swz = np.ascontiguousarray(
            xa.astype(np_dts).reshape(T, 128, XW).transpose(1, 0, 2)
        ).reshape(128, T * XW)
        in_maps.append({
            "xa": xa_swz,
            "xt": xt,
            "crel": np.ascontiguousarray(crel_flat.reshape(T, 128).T)
                      .astype(np_dts),
            "w1": w1c, "w2": w2c, "b1c": b1c, "b2c": b2c, "iota": iota,
            "blkid": blkid, "ident": np.eye(128, dtype=np_dts),
        })
    return in_maps, ntpg, pos


# ================================================================ entry
LAST_RESULTS = None


def kernel(x, batch, W1, b1, W2, b2):
    global LAST_RESULTS
    from concourse.bass_utils import run_bass_kernel_spmd

    in_maps, ntpg, pos = prepare_shards(x, batch, W1, b1, W2, b2)
    key = (ntpg, USE_FP16, USE_XT8, TK)
    if key not in _nc_cache:
        _nc_cache[key] = build_bass(ntpg)
    nc = _nc_cache[key]
    trace = os.environ.get("KERNEL_TRACE", "0") == "1"
    res = run_bass_kernel_spmd(nc, in_maps, core_ids=list(range(N_CORES)),
                               trace=trace)
    LAST_RESULTS = res
    pooled = np.concatenate([r["out"] for r in res.results], axis=0)
    return pooled[pos].astype(np.float32)


# revision 14
# speedup vs baseline: 1.0356x; 1.0098x over previous
"""AttentivePooling Trainium2 kernel (8 NeuronCores, SPMD).

Math (per graph g):  pooled[g] = sum_{n in g} softmax_g(s)_n * x[n]
with s_n = tanh(x W1 + b1) W2 + b2.  Since tanh bounds |s| <= ||W2||_1 + |b2|
(~9 for these inputs), the segment-max subtraction in the reference is
unnecessary: we accumulate  num[g] = sum exp(s_n - SHIFT) x_n  and
den[g] = sum exp(s_n - SHIFT)  in one streaming pass and divide at the end
(the SHIFT cancels).

Sharding: 2048 graphs -> 8 cores x 8 groups x 32 graphs. Graphs are
greedy-bin-packed across the 64 (core,group) bins to balance node counts, so
the common padded capacity C (granularity 256 nodes; partial 2-tile
supertiles handle the tail) stays ~1.5% above the mean; the host undoes the
permutation after the gather. All cores run one identical NEFF. Per 128-node
tile the device:
  - computes h^T = tanh(W1^T x^T + b1); x^T comes from a host-transposed
    fp8(e3m4) copy for nl tiles per chunk (PE upconverts in the matmul; the
    score path tolerates e3m4 on x, rel err ~6e-3) and from on-chip PE
    transposes of the fp16 x for the rest (hybrid balances PE time against
    HBM bytes; fp8 halves the transposed-copy bytes),
  - scores s = h^T.T @ W2 as a [128,1] column per tile, ex = exp(s+b2-SHIFT),
  - builds A[n, j] = ex_n * (iota_j == batch_rel_n) with two chunk-wide
    broadcast-AP tensor_tensor ops,
  - accumulates pooled^T += A.T @ x_aug into a PSUM bank. All pool matmuls
    of a chunk are emitted as one dense block (3-way PE column-tiling makes
    consecutive pool matmuls run ~3x concurrent; interleaving them with
    full-array matmuls would forfeit that concurrency). x_aug's ones column
    yields the denominators; per group the accumulator is reduced/divided/
    stored at group end.
The emission order software-pipelines chunk i's score phase against chunk
i-1's A-build (DVE) and pool block (PE col-masked) so every engine stays fed.
Group-end output DMAs ride the scalar queue so they never head-of-line-block
the sync queue's xa chunk loads. The last group is fully loaded (no on-chip
transposes) so its scores depend only on the early fp8 x^T stream and the
post-last-DMA tail is just A-build + pool + finalize; groups 3-5 carry extra
transposes (TK_HI) to keep PE fed where the DMA stream is the long pole.

Measured on 8 axon trn2 cores: ~314-325us (device-thermal variance), vs
397.6us baseline; rel err ~5.7e-3 (gate 2e-2). Engine occupancy at best run:
PE 92% busy (the long pole), DMA ~85%, ACT ~60%, DVE ~47%.
"""

import os
import sys

for _p in ("/opt/trn_rl_repo",):
    if _p not in sys.path:
        sys.path.insert(0, _p)

import numpy as np

# ---------------------------------------------------------------- geometry
N_NODES = 1048576
HID = 256
HID2 = 128
G_TOTAL = 2048
N_CORES = 8
GT = 32            # graphs per pooling group (PSUM partition dim of pooled)
NG = 8             # groups per core
SEGS_PER_CORE = NG * GT          # 256
XW = HID + 2       # x_aug row width: 256 features + 1.0 + 1 pad zero
ONES_COL = HID     # column index of the ones column
CHUNK_TILES = 44   # 128-node tiles per DMA chunk (2.9 MB @ fp16)
ST = 4             # tiles per score supertile (512 nodes)
TK = int(os.environ.get("KERNEL_TK", "16"))
TK_HI = int(os.environ.get("KERNEL_TK_HI", "20"))
                   # per chunk: last TK tiles transpose x on-chip instead of
                   # loading the fp8 transposed copy. Multiple of ST.

# ---------------------------------------------------------------- dtypes
USE_FP16 = os.environ.get("KERNEL_FP16", "1") == "1"
USE_XT8 = os.environ.get("KERNEL_XT8", "1") == "1"
SHIFT = 8.0 if USE_FP16 else 0.0

_nc_cache = {}


def _dts():
    import concourse.mybir as mybir
    return mybir.dt.float16 if USE_FP16 else mybir.dt.float32


def _np_dts():
    return np.float16 if USE_FP16 else np.float32


def _np_xt_dt():
    if USE_XT8:
        import ml_dtypes
        return ml_dtypes.float8_e3m4
    return _np_dts()


def chunk_schedule(ntpg, ngroups=NG):
    """Shared host/device chunk list: (g, c0, nt, tk, nl, xt_off).

    The first chunk of group 0 and the last chunk of the last group are
    split so the pipeline fills fast (a small all-transposed chunk needs
    only its xa block) and drains fast (the final small chunk is fully
    loaded: no PE transposes on the critical drain path).
    """
    raw = [[(c0, min(CHUNK_TILES, ntpg - c0))
            for c0 in range(0, ntpg, CHUNK_TILES)] for g in range(ngroups)]
    c0, nt = raw[0][0]
    if nt >= 24:
        raw[0] = [(0, 12), (12, nt - 12)] + raw[0][1:]
    c0, nt = raw[-1][-1]
    if nt >= 24:
        raw[-1] = raw[-1][:-1] + [(c0, nt - 12), (c0 + nt - 12, 12)]
    flat = [(g, c0, nt) for g in range(ngroups) for c0, nt in raw[g]]
    out = []
    xt_off = 0
    for i, (g, c0, nt) in enumerate(flat):
        if g == ngroups - 1:
            nl = nt     # drain: the whole last group loads x^T from HBM, so
                        # its scores never wait on late xa and the tail after
                        # the final xa byte is just A-build+pool+finalize
        else:
            tk_want = TK_HI if g in (3, 4, 5) else TK
            # nl multiple of ST so no supertile mixes loaded+transposed
            nl = (max(0, nt - tk_want) // ST) * ST
        tk = nt - nl
        out.append((g, c0, nt, tk, nl, xt_off))
        xt_off += nl * 256
    return out


# ================================================================ device IR
def build_bass(ntpg, ngroups=NG, gt=GT, use_fp16=None):
    """Build + compile the per-core Bass program.

    ntpg: 128-node tiles per group (group capacity C = ntpg*128), mult of 4.
    """
    import concourse.bacc as bacc
    import concourse.mybir as mybir
    import concourse.tile as tile

    if use_fp16 is None:
        use_fp16 = USE_FP16
    dts = mybir.dt.float16 if use_fp16 else mybir.dt.float32
    dtx = mybir.dt.float8e3 if USE_XT8 else dts
    f32 = mybir.dt.float32
    AF = mybir.ActivationFunctionType
    OP = mybir.AluOpType

    assert ntpg % 2 == 0
    T = ngroups * ntpg                  # tiles per core
    S = T * 128                         # padded nodes per core

    nc = bacc.Bacc("TRN2", num_devices=N_CORES)

    sched = chunk_schedule(ntpg, ngroups)
    xtw = sched[-1][5] + sched[-1][4] * 256
    max_nt = max(s[2] for s in sched)

    # xa is host-swizzled partition-major: xa[p, t*XW + d] = x_aug[t*128 + p, d]
    # so any chunk of tiles is a contiguous 2D slice (big DMA runs).
    # xt is packed per chunk: [x^T chunk0 rows | x^T chunk1 rows] per chunk.
    xa = nc.dram_tensor("xa", [128, T * XW], dts, kind="ExternalInput").ap()
    xt = nc.dram_tensor("xt", [128, max(xtw, 256)], dtx,
                        kind="ExternalInput").ap()
    crel = nc.dram_tensor("crel", [128, T], dts, kind="ExternalInput").ap()
    w1 = nc.dram_tensor("w1", [HID, HID2], dts, kind="ExternalInput").ap()
    w2 = nc.dram_tensor("w2", [HID2, 1], dts, kind="ExternalInput").ap()
    b1c = nc.dram_tensor("b1c", [HID2, 1], f32, kind="ExternalInput").ap()
    b2c = nc.dram_tensor("b2c", [128, 1], f32, kind="ExternalInput").ap()
    iota = nc.dram_tensor("iota", [128, max_nt * gt], dts,
                          kind="ExternalInput").ap()
    blkid = nc.dram_tensor("blkid", [128, gt], f32, kind="ExternalInput").ap()
    ident = nc.dram_tensor("ident", [128, 128], dts, kind="ExternalInput").ap()
    out = nc.dram_tensor("out", [ngroups * gt, HID], f32, kind="ExternalOutput").ap()
    n_cg = 3                            # concurrent PE column-groups (PE
                                        # quadrant 3 is buggy; use 0..2)

    with tile.TileContext(nc) as tc:
        with (
            tc.tile_pool(name="consts", bufs=1) as cpool,
            tc.tile_pool(name="xa", bufs=4) as xa_pool,
            tc.tile_pool(name="xt", bufs=3) as xt_pool,
            tc.tile_pool(name="th", bufs=3) as th_pool,
            tc.tile_pool(name="ex", bufs=4) as ex_pool,
            tc.tile_pool(name="amat", bufs=2) as a_pool,
            tc.tile_pool(name="fin", bufs=2) as fin_pool,
            tc.tile_pool(name="xts", bufs=4) as xts_pool,
            tc.tile_pool(name="hp", bufs=2, space="PSUM") as hp_pool,
            tc.tile_pool(name="sp", bufs=1, space="PSUM") as sp_pool,
            tc.tile_pool(name="pp", bufs=2, space="PSUM") as pp_pool,
            tc.tile_pool(name="rp", bufs=1, space="PSUM") as rp_pool,
            tc.tile_pool(name="xtp", bufs=2, space="PSUM") as xtp_pool,
        ):
            # fp16 zeros first: the PE warmup depends only on this memset,
            # so it starts at t~0 and right-sizes to the DMA fill window
            z16_sb = cpool.tile([128, 512], dts)
            nc.gpsimd.memset(z16_sb[:], 0.0)

            # ---- small consts needed first (w1/ident gate the first matmuls)
            w1_sb = cpool.tile([128, 2 * HID2], dts)
            nc.sync.dma_start(out=w1_sb[:, 0:HID2], in_=w1[0:128, :])
            nc.sync.dma_start(out=w1_sb[:, HID2:2 * HID2], in_=w1[128:256, :])
            w2_sb = cpool.tile([128, 1], dts)
            nc.sync.dma_start(out=w2_sb[:], in_=w2[:])
            b1_sb = cpool.tile([128, 1], f32)
            nc.sync.dma_start(out=b1_sb[:], in_=b1c[:])
            b2_sb = cpool.tile([128, 1], f32)
            nc.sync.dma_start(out=b2_sb[:], in_=b2c[:])
            ident_sb = cpool.tile([128, 128], dts)
            nc.sync.dma_start(out=ident_sb[:], in_=ident[:])
            zeros_sb = cpool.tile([128, 512], f32)
            nc.gpsimd.memset(zeros_sb[:], 0.0)

            # larger consts ride behind chunk 0's data (they are only
            # needed by the A-build / group finalize, chunks later)
            iota_sb = cpool.tile([128, max_nt * gt], dts)
            blkid_sb = cpool.tile([128, gt], f32)
            crel_sb = cpool.tile([128, T], dts)

            def emit_late_consts():
                nc.sync.dma_start(out=iota_sb[:], in_=iota[:])
                nc.sync.dma_start(out=blkid_sb[:], in_=blkid[:])
                nc.sync.dma_start(out=crel_sb[:], in_=crel[:])

            # PE warmup: ~4us of fp16 matmuls so the HAM clock-gate opens
            # while the first DMAs land. Sized to the fill window: the old
            # 20x fp32 version (4 cyc/col at low p-state) ran until ~20us
            # and head-of-line-blocked chunk 0's transposes in the in-order
            # PE queue.
            wu_ps = rp_pool.tile([128, 512], f32, space="PSUM", tag="red",
                                 name="wu")

            def emit_warmup():
                for _ in range(8):
                    nc.tensor.matmul(out=wu_ps[:], lhsT=z16_sb[:, 0:128],
                                     rhs=z16_sb[:], start=True, stop=True,
                                     skip_group_check=True)

            # software pipeline: chunk i's score phase is emitted interleaved
            # with chunk i-1's A-build (DVE), then chunk i-1's pool matmuls
            # follow as one dense col-tiled block.
            chunks = [(g, c0, nt) for g, c0, nt, tk, nl, xo in sched]

            state = {}       # chunk idx -> dict with tiles needed by pool
            group_ps = {}    # group -> pool accumulator

            def emit_dmas(i):
                g, c0, nt, tk, nl, xo = sched[i]
                t0_abs = g * ntpg + c0
                xa_sb = xa_pool.tile([128, nt * XW], dts, tag="xa")
                nc.sync.dma_start(
                    out=xa_sb[:], in_=xa[:, t0_abs * XW:(t0_abs + nt) * XW])
                st_ = {"xa": xa_sb, "nl": nl, "t0_abs": t0_abs, "g": g,
                       "c0": c0, "nt": nt}
                if nl:
                    xtb_sb = xt_pool.tile([128, nl * 256], dtx, tag="xtb")
                    nc.gpsimd.dma_start(
                        out=xtb_sb[:], in_=xt[:, xo:xo + nl * 256])
                    st_["xt0"] = xtb_sb[:, 0:nl * 128]
                    st_["xt1"] = xtb_sb[:, nl * 128:nl * 256]
                st_["sp"] = sp_pool.tile([128, nt], f32, space="PSUM", tag="sp",
                                         name="sp")
                state[i] = st_

            def score_ops(i):
                g, c0, nt = chunks[i]
                st_ = state[i]
                xa_sb, nl, sp = st_.get("xa"), st_["nl"], st_["sp"]

                def one_supertile(st):
                    k = min(ST, nt - st * ST)   # partial tail supertile ok
                    w = k * 128
                    hp = hp_pool.tile([128, w], f32, space="PSUM", tag="hp")
                    if st * ST >= nl:
                        # one PSUM bank holds the supertile's 2k trans-
                        # posed [128,128] blocks, laid out half-major so the
                        # h matmuls read two contiguous N=w slices
                        xtp = xtp_pool.tile([128, 2 * w], dts,
                                            space="PSUM", tag="xtp")
                        for pr in range(k // 2):
                            t_lo = st * ST + pr * 2
                            for u in range(2):
                                for c in range(2):
                                    o = c * w + (pr * 2 + u) * 128
                                    nc.tensor.transpose(
                                        out=xtp[:, o:o + 128],
                                        in_=xa_sb[:, (t_lo + u) * XW + c * 128:
                                                  (t_lo + u) * XW + (c + 1) * 128],
                                        identity=ident_sb[:])
                        xts = xts_pool.tile([128, 2 * w], dts, tag="xts")
                        # PSUM->SBUF copies split DVE:ACT ~5:1 (DVE copy
                        # ~850ns, ACT copy ~1200ns; ACT is tanh-heavy)
                        if (st - (nl // ST)) % 6 < 5:
                            nc.vector.tensor_copy(xts[:], xtp[:])
                        else:
                            nc.scalar.copy(xts[:], xtp[:])
                        rhs0, rhs1 = xts[:, 0:w], xts[:, w:2 * w]
                    else:
                        rhs0 = st_["xt0"][:, st * ST * 128:st * ST * 128 + w]
                        rhs1 = st_["xt1"][:, st * ST * 128:st * ST * 128 + w]
                    # w2 scores lag one supertile (tanh long done) and sit
                    # between the transposes and the copy-dependent h matmuls
                    # so the in-order PE queue has ready work during the
                    # PSUM->SBUF copy
                    if st > 0:
                        w2_block(st - 1)
                    nc.tensor.matmul(
                        out=hp[:], lhsT=w1_sb[:, 0:HID2],
                        rhs=rhs0, start=True, stop=False)
                    nc.tensor.matmul(
                        out=hp[:], lhsT=w1_sb[:, HID2:2 * HID2],
                        rhs=rhs1, start=False, stop=True)
                    th = th_pool.tile([128, w], dts, tag="th")
                    nc.scalar.activation(th[:], hp[:], AF.Tanh,
                                         bias=b1_sb[:, 0:1])
                    st_.setdefault("th", {})[st] = th

                def w2_block(st):
                    th = st_["th"].pop(st)
                    for j in range(min(ST, nt - st * ST)):
                        jj = st * ST + j
                        nc.tensor.matmul(
                            out=sp[:, jj:jj + 1],
                            lhsT=th[:, j * 128:(j + 1) * 128],
                            rhs=w2_sb[:],
                            start=(jj == 0), stop=(jj == nt - 1),
                            skip_group_check=True)

                def fin():
                    w2_block((nt + ST - 1) // ST - 1)
                    ex = ex_pool.tile([128, nt], dts, tag="ex")
                    nc.scalar.activation(ex[:], sp[:], AF.Exp,
                                         bias=b2_sb[:, 0:1])
                    st_["ex"] = ex

                return [lambda st=st: one_supertile(st)
                        for st in range((nt + ST - 1) // ST)] + [fin]

            def a4_ops(i):
                """Chunk-wide A-matrix build: 2 DVE ops over [128, nt*gt]."""
                g, c0, nt = chunks[i]
                st_ = state[i]
                t0_abs = st_["t0_abs"]

                def build_eq():
                    a4 = a_pool.tile([128, nt * gt], dts, tag="a4")
                    st_["a4"] = a4
                    nc.vector.tensor_tensor(
                        out=a4[:].rearrange("p (t o) -> p t o", o=gt),
                        in0=iota_sb[:, 0:nt * gt].rearrange(
                            "p (t o) -> p t o", o=gt),
                        in1=crel_sb[:, t0_abs:t0_abs + nt].broadcast_to(
                            [128, nt, gt]),
                        op=OP.is_equal)

                def build_mul():
                    a4 = st_["a4"]
                    a4v = a4[:].rearrange("p (t o) -> p t o", o=gt)
                    nc.vector.tensor_tensor(
                        out=a4v, in0=a4v,
                        in1=st_["ex"][:].broadcast_to([128, nt, gt]),
                        op=OP.mult)

                return [build_eq, build_mul]

            def pool_block(i):
                """Dense col-tiled pool matmul block for chunk i."""
                g, c0, nt = chunks[i]
                st_ = state[i]
                xa_sb = st_["xa"]

                def run():
                    if c0 == 0:
                        pool_ps = pp_pool.tile([128, 512], f32, space="PSUM",
                                               tag="pool")
                        group_ps[g] = pool_ps
                        nc.tensor.matmul(
                            out=pool_ps[:], lhsT=z16_sb[:, 0:128],
                            rhs=z16_sb[:],
                            start=True, stop=False, skip_group_check=True)
                    pool_ps = group_ps[g]
                    a4 = st_["a4"]
                    for j in range(nt):
                        t_in_g = c0 + j
                        a = t_in_g % n_cg
                        nc.tensor.matmul(
                            out=pool_ps[gt * a:gt * (a + 1), 0:XW],
                            lhsT=a4[:, j * gt:(j + 1) * gt],
                            rhs=xa_sb[:, j * XW:(j + 1) * XW],
                            start=False, stop=(t_in_g == ntpg - 1),
                            tile_position=(0, gt * a),
                            skip_group_check=True)
                    if c0 + nt >= ntpg:
                        pool_ps = group_ps.pop(g)
                        acc_sb = fin_pool.tile([128, XW], f32, tag="acc")
                        nc.vector.tensor_copy(acc_sb[:], pool_ps[:, 0:XW])
                        red_ps = rp_pool.tile([gt, XW], f32, space="PSUM",
                                              tag="red")
                        nc.tensor.matmul(out=red_ps[:], lhsT=blkid_sb[:],
                                         rhs=acc_sb[:], start=True, stop=True)
                        rec = fin_pool.tile([gt, 1], f32, tag="rec")
                        nc.vector.reciprocal(
                            rec[:], red_ps[:, ONES_COL:ONES_COL + 1])
                        og = fin_pool.tile([gt, HID], f32, tag="og")
                        nc.vector.tensor_scalar(
                            og[:], red_ps[:, 0:HID], rec[:, 0:1], None, OP.mult)
                        # scalar queue: never head-of-line-blocks xa loads
                        nc.scalar.dma_start(out=out[g * gt:(g + 1) * gt, :],
                                            in_=og[:])
                    del state[i]

                return run

            emit_warmup()
            emit_dmas(0)
            emit_late_consts()
            for i in range(len(chunks) + 1):
                s_ops = score_ops(i) if i < len(chunks) else []
                a_ops = a4_ops(i - 1) if i > 0 else []
                p_run = pool_block(i - 1) if i > 0 else None
                k = max(len(s_ops), len(a_ops))
                for q in range(k):
                    if q < len(s_ops):
                        s_ops[q]()
                    if q == 0 and i + 1 < len(chunks):
                        emit_dmas(i + 1)
                    if q < len(a_ops):
                        a_ops[q]()
                if p_run is not None:
                    p_run()

    nc.compile()
    return nc


# ================================================================ host prep
def pack_groups(counts, n_bins, gt):
    """Greedy bin-packing: graphs -> bins of exactly gt graphs, balancing
    node load so the padded group capacity C shrinks. Returns
    (graphs_of_bin[b] lists, slot_of_graph)."""
    import heapq
    order = np.argsort(-counts, kind="stable")
    load = np.zeros(n_bins, np.int64)
    members = [[] for _ in range(n_bins)]
    h = [(0, b) for b in range(n_bins)]
    heapq.heapify(h)
    for gid in order:
        popped = []
        while True:
            l, b = heapq.heappop(h)
            if len(members[b]) < gt:
                break
            popped.append((l, b))
        for p in popped:
            heapq.heappush(h, p)
        members[b].append(int(gid))
        load[b] += int(counts[gid])
        if len(members[b]) < gt:
            heapq.heappush(h, (int(load[b]), b))
    slot = np.zeros(len(counts), np.int64)
    for b in range(n_bins):
        for s, gid in enumerate(members[b]):
            slot[gid] = s
    return members, slot, int(load.max())


def prepare_shards(x, batch, W1, b1, W2, b2, ngroups=NG, gt=GT, n_cores=N_CORES):
    """Split nodes into (core, group) node blocks padded to capacity C."""
    np_dts = _np_dts()
    np_xt = _np_xt_dt()
    x = np.asarray(x)
    batch = np.asarray(batch).astype(np.int64)
    g_total = n_cores * ngroups * gt
    counts = np.bincount(batch, minlength=g_total)
    starts = np.concatenate([[0], np.cumsum(counts)])[:-1]
    n_bins = n_cores * ngroups
    members, slot, max_load = pack_groups(counts, n_bins, gt)
    # C granularity is 2 tiles (ST even): partial supertiles handle the tail
    C = int(max(512, ((max_load + 255) // 256) * 256))
    ntpg = C // 128
    T = ngroups * ntpg

    sched = chunk_schedule(ntpg, ngroups)
    max_nt = max(s[2] for s in sched)

    w1c = np.ascontiguousarray(W1).astype(np_dts)
    w2c = np.ascontiguousarray(W2).astype(np_dts)
    b1c = np.asarray(b1, np.float32).reshape(HID2, 1)
    b2c = np.full((128, 1), float(np.asarray(b2).reshape(-1)[0]) - SHIFT,
                  np.float32)
    iota = np.tile(np.arange(gt, dtype=np.float32), (128, max_nt)).astype(np_dts)
    blkid = np.zeros((128, gt), np.float32)
    blkid[np.arange(128), np.arange(128) % gt] = 1.0

    # out row for graph gid: core*ngroups*gt + group*gt + slot
    pos = np.zeros(g_total, np.int64)
    in_maps = []
    for core in range(n_cores):
        xa = np.zeros((ngroups * C, XW), np.float32)
        crel_flat = np.full(ngroups * C, -1.0, np.float32)
        for g in range(ngroups):
            b = core * ngroups + g
            off = g * C
            for s, gid in enumerate(members[b]):
                s0, n = int(starts[gid]), int(counts[gid])
                xa[off:off + n, :HID] = x[s0:s0 + n]
                crel_flat[off:off + n] = float(s)
                pos[gid] = (core * ngroups + g) * gt + s
                off += n
        xa[:, ONES_COL] = 1.0
        xtfull = np.ascontiguousarray(xa[:, :HID].T).astype(np_xt)
        # pack x^T per chunk (both hidden halves back to back, contiguous)
        xtw = sched[-1][5] + sched[-1][4] * 256
        xt = np.zeros((128, max(xtw, 256)), np_xt)
        for g, c0, nt, tk, nl, xo in sched:
            n0 = (g * ntpg + c0) * 128
            xt[:, xo:xo + nl * 128] = xtfull[0:128, n0:n0 + nl * 128]
            xt[:, xo + nl * 128:xo + nl * 256] = xtfull[128:256, n0:n0 + nl * 128]
        # partition-major swizzle: xa_swz[p, t*XW + d] = xa[t*128 + p, d]
        xa_swz = np.ascontiguousarray(
            xa.astype(np_dts).reshape(T, 128, XW).transpose(1, 0, 2)
        ).reshape(128, T * XW)
        in_maps.append({
            "xa": xa_swz,
            "xt": xt,
            "crel": np.ascontiguousarray(crel_flat.reshape(T, 128).T)
                      .astype(np_dts),
            "w1": w1c, "w2": w2c, "b1c": b1c, "b2c": b2c, "iota": iota,
            "blkid": blkid, "ident": np.eye(128, dtype=np_dts),
        })
    return in_maps, ntpg, pos


# ================================================================ entry
LAST_RESULTS = None


def kernel(x, batch, W1, b1, W2, b2):
    global LAST_RESULTS
    from concourse.bass_utils import run_bass_kernel_spmd

    in_maps, ntpg, pos = prepare_shards(x, batch, W1, b1, W2, b2)
    key = (ntpg, USE_FP16, USE_XT8, TK)
    if key not in _nc_cache:
        _nc_cache[key] = build_bass(ntpg)
    nc = _nc_cache[key]
    trace = os.environ.get("KERNEL_TRACE", "0") == "1"
    res = run_bass_kernel_spmd(nc, in_maps, core_ids=list(range(N_CORES)),
                               trace=trace)
    LAST_RESULTS = res
    pooled = np.concatenate([r["out"] for r in res.results], axis=0)
    return pooled[pos].astype(np.float32)


# revision 16
# speedup vs baseline: 1.0506x; 1.0145x over previous
"""AttentivePooling Trainium2 kernel (8 NeuronCores, SPMD).

Math (per graph g):  pooled[g] = sum_{n in g} softmax_g(s)_n * x[n]
with s_n = tanh(x W1 + b1) W2 + b2.  Since tanh bounds |s| <= ||W2||_1 + |b2|
(~9 for these inputs), the segment-max subtraction in the reference is
unnecessary: we accumulate  num[g] = sum exp(s_n - SHIFT) x_n  and
den[g] = sum exp(s_n - SHIFT)  in one streaming pass and divide at the end
(the SHIFT cancels).

Sharding: 2048 graphs -> 8 cores x 8 groups x 32 graphs. Graphs are
greedy-bin-packed across the 64 (core,group) bins to balance node counts, so
the common padded capacity C (granularity 256 nodes; partial 2-tile
supertiles handle the tail) stays ~1.5% above the mean; the host undoes the
permutation after the gather. All cores run one identical NEFF. Per 128-node
tile the device:
  - computes h^T = tanh(W1^T x^T + b1); x^T comes from a host-transposed
    fp8(e3m4) copy for nl tiles per chunk (PE upconverts in the matmul; the
    score path tolerates e3m4 on x, rel err ~6e-3) and from on-chip PE
    transposes of the fp16 x for the rest (hybrid balances PE time against
    HBM bytes; fp8 halves the transposed-copy bytes),
  - scores s = h^T.T @ W2 as a [128,1] column per tile, ex = exp(s+b2-SHIFT),
  - builds A[n, j] = ex_n * (iota_j == batch_rel_n) with two chunk-wide
    broadcast-AP tensor_tensor ops,
  - accumulates pooled^T += A.T @ x_aug into a PSUM bank. All pool matmuls
    of a chunk are emitted as one dense block (3-way PE column-tiling makes
    consecutive pool matmuls run ~3x concurrent; interleaving them with
    full-array matmuls would forfeit that concurrency). x_aug's ones column
    yields the denominators; per group the accumulator is reduced/divided/
    stored at group end.
The emission order software-pipelines chunk i's score phase against chunk
i-1's A-build (DVE) and pool block (PE col-masked) so every engine stays fed.
Group-end output DMAs ride the scalar queue so they never head-of-line-block
the sync queue's xa chunk loads. The last group is fully loaded (no on-chip
transposes) so its scores depend only on the early fp8 x^T stream and the
post-last-DMA tail is just A-build + pool + finalize; groups 3-5 carry extra
transposes (TK_HI) to keep PE fed where the DMA stream is the long pole.

Measured on 8 axon trn2 cores: ~314-325us (device-thermal variance), vs
397.6us baseline; rel err ~5.7e-3 (gate 2e-2). Engine occupancy at best run:
PE 92% busy (the long pole), DMA ~85%, ACT ~60%, DVE ~47%.
"""

import os
import sys

for _p in ("/opt/trn_rl_repo",):
    if _p not in sys.path:
        sys.path.insert(0, _p)

import numpy as np

# ---------------------------------------------------------------- geometry
N_NODES = 1048576
HID = 256
HID2 = 128
G_TOTAL = 2048
N_CORES = 8
GT = 32            # graphs per pooling group (PSUM partition dim of pooled)
NG = 8             # groups per core
SEGS_PER_CORE = NG * GT          # 256
XW = HID + 2       # x_aug row width: 256 features + 1.0 + 1 pad zero
ONES_COL = HID     # column index of the ones column
CHUNK_TILES = 44   # 128-node tiles per DMA chunk (2.9 MB @ fp16)
ST = 4             # tiles per score supertile (512 nodes)
TK = int(os.environ.get("KERNEL_TK", "16"))
TK_HI = int(os.environ.get("KERNEL_TK_HI", "20"))
                   # per chunk: last TK tiles transpose x on-chip instead of
                   # loading the fp8 transposed copy. Multiple of ST.

# ---------------------------------------------------------------- dtypes
USE_FP16 = os.environ.get("KERNEL_FP16", "1") == "1"
USE_XT8 = os.environ.get("KERNEL_XT8", "1") == "1"
SHIFT = 8.0 if USE_FP16 else 0.0

_nc_cache = {}


def _dts():
    import concourse.mybir as mybir
    return mybir.dt.float16 if USE_FP16 else mybir.dt.float32


def _np_dts():
    return np.float16 if USE_FP16 else np.float32


def _np_xt_dt():
    if USE_XT8:
        import ml_dtypes
        return ml_dtypes.float8_e3m4
    return _np_dts()


def chunk_schedule(ntpg, ngroups=NG):
    """Shared host/device chunk list: (g, c0, nt, tk, nl, xt_off).

    The first chunk of group 0 and the last chunk of the last group are
    split so the pipeline fills fast (a small all-transposed chunk needs
    only its xa block) and drains fast (the final small chunk is fully
    loaded: no PE transposes on the critical drain path).
    """
    raw = [[(c0, min(CHUNK_TILES, ntpg - c0))
            for c0 in range(0, ntpg, CHUNK_TILES)] for g in range(ngroups)]
    c0, nt = raw[0][0]
    if nt >= 24:
        raw[0] = [(0, 12), (12, nt - 12)] + raw[0][1:]
    c0, nt = raw[-1][-1]
    if nt >= 24:
        raw[-1] = raw[-1][:-1] + [(c0, nt - 12), (c0 + nt - 12, 12)]
    flat = [(g, c0, nt) for g in range(ngroups) for c0, nt in raw[g]]
    out = []
    xt_off = 0
    for i, (g, c0, nt) in enumerate(flat):
        if g == ngroups - 1:
            nl = nt     # drain: the whole last group loads x^T from HBM, so
                        # its scores never wait on late xa and the tail after
                        # the final xa byte is just A-build+pool+finalize
        else:
            tk_want = TK_HI if g in (3, 4, 5) else TK
            # nl multiple of ST so no supertile mixes loaded+transposed
            nl = (max(0, nt - tk_want) // ST) * ST
        tk = nt - nl
        out.append((g, c0, nt, tk, nl, xt_off))
        xt_off += nl * 256
    return out


# ================================================================ device IR
def build_bass(ntpg, ngroups=NG, gt=GT, use_fp16=None):
    """Build + compile the per-core Bass program.

    ntpg: 128-node tiles per group (group capacity C = ntpg*128), mult of 4.
    """
    import concourse.bacc as bacc
    import concourse.mybir as mybir
    import concourse.tile as tile

    if use_fp16 is None:
        use_fp16 = USE_FP16
    dts = mybir.dt.float16 if use_fp16 else mybir.dt.float32
    dtx = mybir.dt.float8e3 if USE_XT8 else dts
    f32 = mybir.dt.float32
    AF = mybir.ActivationFunctionType
    OP = mybir.AluOpType

    assert ntpg % 2 == 0
    T = ngroups * ntpg                  # tiles per core
    S = T * 128                         # padded nodes per core

    nc = bacc.Bacc("TRN2", num_devices=N_CORES)

    sched = chunk_schedule(ntpg, ngroups)
    xtw = sched[-1][5] + sched[-1][4] * 256
    max_nt = max(s[2] for s in sched)

    # xa is host-swizzled partition-major: xa[p, t*XW + d] = x_aug[t*128 + p, d]
    # so any chunk of tiles is a contiguous 2D slice (big DMA runs).
    # xt is packed per chunk: [x^T chunk0 rows | x^T chunk1 rows] per chunk.
    xa = nc.dram_tensor("xa", [128, T * XW], dts, kind="ExternalInput").ap()
    xt = nc.dram_tensor("xt", [128, max(xtw, 256)], dtx,
                        kind="ExternalInput").ap()
    crel = nc.dram_tensor("crel", [128, T], dts, kind="ExternalInput").ap()
    # all small consts ride in two packed tensors (one DMA each): five
    # separate 256B-per-partition DMAs were descriptor-bound (~10us) and
    # head-of-line-blocked the first xa chunk on the sync queue
    ck16 = nc.dram_tensor("ck16", [128, 2 * HID2 + 128 + 1], dts,
                          kind="ExternalInput").ap()
    ck32 = nc.dram_tensor("ck32", [128, 2 + gt], f32,
                          kind="ExternalInput").ap()
    iota = nc.dram_tensor("iota", [128, max_nt * gt], dts,
                          kind="ExternalInput").ap()
    out = nc.dram_tensor("out", [ngroups * gt, HID], f32, kind="ExternalOutput").ap()
    n_cg = 3                            # concurrent PE column-groups (PE
                                        # quadrant 3 is buggy; use 0..2)

    with tile.TileContext(nc) as tc:
        with (
            tc.tile_pool(name="consts", bufs=1) as cpool,
            tc.tile_pool(name="xa", bufs=4) as xa_pool,
            tc.tile_pool(name="xt", bufs=3) as xt_pool,
            tc.tile_pool(name="th", bufs=3) as th_pool,
            tc.tile_pool(name="ex", bufs=4) as ex_pool,
            tc.tile_pool(name="amat", bufs=2) as a_pool,
            tc.tile_pool(name="fin", bufs=2) as fin_pool,
            tc.tile_pool(name="xts", bufs=4) as xts_pool,
            tc.tile_pool(name="hp", bufs=2, space="PSUM") as hp_pool,
            tc.tile_pool(name="sp", bufs=1, space="PSUM") as sp_pool,
            tc.tile_pool(name="pp", bufs=2, space="PSUM") as pp_pool,
            tc.tile_pool(name="rp", bufs=1, space="PSUM") as rp_pool,
            tc.tile_pool(name="xtp", bufs=2, space="PSUM") as xtp_pool,
        ):
            # fp16 zeros first: the PE warmup depends only on this memset,
            # so it starts at t~0 and right-sizes to the DMA fill window
            z16_sb = cpool.tile([128, 512], dts)
            nc.gpsimd.memset(z16_sb[:], 0.0)

            # ---- packed consts: one fp16 DMA (w1 | ident | w2) and one
            # f32 DMA (b1 | b2 | blkid); views alias the packed tiles
            ck16_sb = cpool.tile([128, 2 * HID2 + 128 + 1], dts)
            nc.sync.dma_start(out=ck16_sb[:], in_=ck16[:])
            w1_sb = ck16_sb[:, 0:2 * HID2]
            ident_sb = ck16_sb[:, 2 * HID2:2 * HID2 + 128]
            w2_sb = ck16_sb[:, 2 * HID2 + 128:2 * HID2 + 129]
            ck32_sb = cpool.tile([128, 2 + gt], f32)
            nc.sync.dma_start(out=ck32_sb[:], in_=ck32[:])
            b1_sb = ck32_sb[:, 0:1]
            b2_sb = ck32_sb[:, 1:2]
            blkid_sb = ck32_sb[:, 2:2 + gt]
            zeros_sb = cpool.tile([128, 512], f32)
            nc.gpsimd.memset(zeros_sb[:], 0.0)

            # larger consts ride behind chunk 0's data (they are only
            # needed by the A-build / group finalize, chunks later)
            iota_sb = cpool.tile([128, max_nt * gt], dts)
            crel_sb = cpool.tile([128, T], dts)

            def emit_late_consts():
                nc.sync.dma_start(out=iota_sb[:], in_=iota[:])
                nc.sync.dma_start(out=crel_sb[:], in_=crel[:])

            # PE warmup: ~4us of fp16 matmuls so the HAM clock-gate opens
            # while the first DMAs land. Sized to the fill window: the old
            # 20x fp32 version (4 cyc/col at low p-state) ran until ~20us
            # and head-of-line-blocked chunk 0's transposes in the in-order
            # PE queue.
            wu_ps = rp_pool.tile([128, 512], f32, space="PSUM", tag="red",
                                 name="wu")

            def emit_warmup():
                for _ in range(8):
                    nc.tensor.matmul(out=wu_ps[:], lhsT=z16_sb[:, 0:128],
                                     rhs=z16_sb[:], start=True, stop=True,
                                     skip_group_check=True)

            # software pipeline: chunk i's score phase is emitted interleaved
            # with chunk i-1's A-build (DVE), then chunk i-1's pool matmuls
            # follow as one dense col-tiled block.
            chunks = [(g, c0, nt) for g, c0, nt, tk, nl, xo in sched]

            state = {}       # chunk idx -> dict with tiles needed by pool
            group_ps = {}    # group -> pool accumulator

            def emit_dmas(i):
                g, c0, nt, tk, nl, xo = sched[i]
                t0_abs = g * ntpg + c0
                xa_sb = xa_pool.tile([128, nt * XW], dts, tag="xa")
                nc.sync.dma_start(
                    out=xa_sb[:], in_=xa[:, t0_abs * XW:(t0_abs + nt) * XW])
                st_ = {"xa": xa_sb, "nl": nl, "t0_abs": t0_abs, "g": g,
                       "c0": c0, "nt": nt}
                if nl:
                    xtb_sb = xt_pool.tile([128, nl * 256], dtx, tag="xtb")
                    nc.gpsimd.dma_start(
                        out=xtb_sb[:], in_=xt[:, xo:xo + nl * 256])
                    st_["xt0"] = xtb_sb[:, 0:nl * 128]
                    st_["xt1"] = xtb_sb[:, nl * 128:nl * 256]
                st_["sp"] = sp_pool.tile([128, nt], f32, space="PSUM", tag="sp",
                                         name="sp")
                state[i] = st_

            def score_ops(i):
                g, c0, nt = chunks[i]
                st_ = state[i]
                xa_sb, nl, sp = st_.get("xa"), st_["nl"], st_["sp"]

                def one_supertile(st):
                    k = min(ST, nt - st * ST)   # partial tail supertile ok
                    w = k * 128
                    hp = hp_pool.tile([128, w], f32, space="PSUM", tag="hp")
                    if st * ST >= nl:
                        # one PSUM bank holds the supertile's 2k trans-
                        # posed [128,128] blocks, laid out half-major so the
                        # h matmuls read two contiguous N=w slices
                        xtp = xtp_pool.tile([128, 2 * w], dts,
                                            space="PSUM", tag="xtp")
                        for pr in range(k // 2):
                            t_lo = st * ST + pr * 2
                            for u in range(2):
                                for c in range(2):
                                    o = c * w + (pr * 2 + u) * 128
                                    nc.tensor.transpose(
                                        out=xtp[:, o:o + 128],
                                        in_=xa_sb[:, (t_lo + u) * XW + c * 128:
                                                  (t_lo + u) * XW + (c + 1) * 128],
                                        identity=ident_sb)
                        xts = xts_pool.tile([128, 2 * w], dts, tag="xts")
                        # PSUM->SBUF copies split DVE:ACT ~5:1 (DVE copy
                        # ~850ns, ACT copy ~1200ns; ACT is tanh-heavy)
                        if (st - (nl // ST)) % 6 < 5:
                            nc.vector.tensor_copy(xts[:], xtp[:])
                        else:
                            nc.scalar.copy(xts[:], xtp[:])
                        rhs0, rhs1 = xts[:, 0:w], xts[:, w:2 * w]
                    else:
                        rhs0 = st_["xt0"][:, st * ST * 128:st * ST * 128 + w]
                        rhs1 = st_["xt1"][:, st * ST * 128:st * ST * 128 + w]
                    # w2 scores lag one supertile (tanh long done) and sit
                    # between the transposes and the copy-dependent h matmuls
                    # so the in-order PE queue has ready work during the
                    # PSUM->SBUF copy
                    if st > 0:
                        w2_block(st - 1)
                    nc.tensor.matmul(
                        out=hp[:], lhsT=w1_sb[:, 0:HID2],
                        rhs=rhs0, start=True, stop=False)
                    nc.tensor.matmul(
                        out=hp[:], lhsT=w1_sb[:, HID2:2 * HID2],
                        rhs=rhs1, start=False, stop=True)
                    th = th_pool.tile([128, w], dts, tag="th")
                    nc.scalar.activation(th[:], hp[:], AF.Tanh,
                                         bias=b1_sb)
                    st_.setdefault("th", {})[st] = th

                def w2_block(st):
                    th = st_["th"].pop(st)
                    for j in range(min(ST, nt - st * ST)):
                        jj = st * ST + j
                        nc.tensor.matmul(
                            out=sp[:, jj:jj + 1],
                            lhsT=th[:, j * 128:(j + 1) * 128],
                            rhs=w2_sb,
                            start=(jj == 0), stop=(jj == nt - 1),
                            skip_group_check=True)

                def fin():
                    w2_block((nt + ST - 1) // ST - 1)
                    ex = ex_pool.tile([128, nt], dts, tag="ex")
                    nc.scalar.activation(ex[:], sp[:], AF.Exp,
                                         bias=b2_sb)
                    st_["ex"] = ex

                return [lambda st=st: one_supertile(st)
                        for st in range((nt + ST - 1) // ST)] + [fin]

            def a4_ops(i):
                """Chunk-wide A-matrix build: 2 DVE ops over [128, nt*gt]."""
                g, c0, nt = chunks[i]
                st_ = state[i]
                t0_abs = st_["t0_abs"]

                def build_eq():
                    a4 = a_pool.tile([128, nt * gt], dts, tag="a4")
                    st_["a4"] = a4
                    nc.vector.tensor_tensor(
                        out=a4[:].rearrange("p (t o) -> p t o", o=gt),
                        in0=iota_sb[:, 0:nt * gt].rearrange(
                            "p (t o) -> p t o", o=gt),
                        in1=crel_sb[:, t0_abs:t0_abs + nt].broadcast_to(
                            [128, nt, gt]),
                        op=OP.is_equal)

                def build_mul():
                    a4 = st_["a4"]
                    a4v = a4[:].rearrange("p (t o) -> p t o", o=gt)
                    nc.vector.tensor_tensor(
                        out=a4v, in0=a4v,
                        in1=st_["ex"][:].broadcast_to([128, nt, gt]),
                        op=OP.mult)

                return [build_eq, build_mul]

            def pool_block(i):
                """Dense col-tiled pool matmul block for chunk i."""
                g, c0, nt = chunks[i]
                st_ = state[i]
                xa_sb = st_["xa"]

                def run():
                    if c0 == 0:
                        pool_ps = pp_pool.tile([128, 512], f32, space="PSUM",
                                               tag="pool")
                        group_ps[g] = pool_ps
                        nc.tensor.matmul(
                            out=pool_ps[:], lhsT=z16_sb[:, 0:128],
                            rhs=z16_sb[:],
                            start=True, stop=False, skip_group_check=True)
                    pool_ps = group_ps[g]
                    a4 = st_["a4"]
                    for j in range(nt):
                        t_in_g = c0 + j
                        a = t_in_g % n_cg
                        nc.tensor.matmul(
                            out=pool_ps[gt * a:gt * (a + 1), 0:XW],
                            lhsT=a4[:, j * gt:(j + 1) * gt],
                            rhs=xa_sb[:, j * XW:(j + 1) * XW],
                            start=False, stop=(t_in_g == ntpg - 1),
                            tile_position=(0, gt * a),
                            skip_group_check=True)
                    if c0 + nt >= ntpg:
                        pool_ps = group_ps.pop(g)
                        acc_sb = fin_pool.tile([128, XW], f32, tag="acc")
                        nc.vector.tensor_copy(acc_sb[:], pool_ps[:, 0:XW])
                        red_ps = rp_pool.tile([gt, XW], f32, space="PSUM",
                                              tag="red")
                        nc.tensor.matmul(out=red_ps[:], lhsT=blkid_sb,
                                         rhs=acc_sb[:], start=True, stop=True)
                        rec = fin_pool.tile([gt, 1], f32, tag="rec")
                        nc.vector.reciprocal(
                            rec[:], red_ps[:, ONES_COL:ONES_COL + 1])
                        og = fin_pool.tile([gt, HID], f32, tag="og")
                        nc.vector.tensor_scalar(
                            og[:], red_ps[:, 0:HID], rec[:, 0:1], None, OP.mult)
                        # scalar queue: never head-of-line-blocks xa loads
                        nc.scalar.dma_start(out=out[g * gt:(g + 1) * gt, :],
                                            in_=og[:])
                    del state[i]

                return run

            emit_warmup()
            emit_dmas(0)
            emit_late_consts()
            for i in range(len(chunks) + 1):
                s_ops = score_ops(i) if i < len(chunks) else []
                a_ops = a4_ops(i - 1) if i > 0 else []
                p_run = pool_block(i - 1) if i > 0 else None
                k = max(len(s_ops), len(a_ops))
                for q in range(k):
                    if q < len(s_ops):
                        s_ops[q]()
                    if q == 0 and i + 1 < len(chunks):
                        emit_dmas(i + 1)
                    if q < len(a_ops):
                        a_ops[q]()
                if p_run is not None:
                    p_run()

    nc.compile()
    return nc


# ================================================================ host prep
def pack_groups(counts, n_bins, gt):
    """Greedy bin-packing: graphs -> bins of exactly gt graphs, balancing
    node load so the padded group capacity C shrinks. Returns
    (graphs_of_bin[b] lists, slot_of_graph)."""
    import heapq
    order = np.argsort(-counts, kind="stable")
    load = np.zeros(n_bins, np.int64)
    members = [[] for _ in range(n_bins)]
    h = [(0, b) for b in range(n_bins)]
    heapq.heapify(h)
    for gid in order:
        popped = []
        while True:
            l, b = heapq.heappop(h)
            if len(members[b]) < gt:
                break
            popped.append((l, b))
        for p in popped:
            heapq.heappush(h, p)
        members[b].append(int(gid))
        load[b] += int(counts[gid])
        if len(members[b]) < gt:
            heapq.heappush(h, (int(load[b]), b))
    slot = np.zeros(len(counts), np.int64)
    for b in range(n_bins):
        for s, gid in enumerate(members[b]):
            slot[gid] = s
    return members, slot, int(load.max())


def prepare_shards(x, batch, W1, b1, W2, b2, ngroups=NG, gt=GT, n_cores=N_CORES):
    """Split nodes into (core, group) node blocks padded to capacity C."""
    np_dts = _np_dts()
    np_xt = _np_xt_dt()
    x = np.asarray(x)
    batch = np.asarray(batch).astype(np.int64)
    g_total = n_cores * ngroups * gt
    counts = np.bincount(batch, minlength=g_total)
    starts = np.concatenate([[0], np.cumsum(counts)])[:-1]
    n_bins = n_cores * ngroups
    members, slot, max_load = pack_groups(counts, n_bins, gt)
    # C granularity is 2 tiles (ST even): partial supertiles handle the tail
    C = int(max(512, ((max_load + 255) // 256) * 256))
    ntpg = C // 128
    T = ngroups * ntpg

    sched = chunk_schedule(ntpg, ngroups)
    max_nt = max(s[2] for s in sched)

    w1c = np.ascontiguousarray(W1).astype(np_dts)
    w2c = np.ascontiguousarray(W2).astype(np_dts)
    b1c = np.asarray(b1, np.float32).reshape(HID2, 1)
    b2c = np.full((128, 1), float(np.asarray(b2).reshape(-1)[0]) - SHIFT,
                  np.float32)
    iota = np.tile(np.arange(gt, dtype=np.float32), (128, max_nt)).astype(np_dts)
    blkid = np.zeros((128, gt), np.float32)
    blkid[np.arange(128), np.arange(128) % gt] = 1.0
    ck16 = np.concatenate(
        [w1c[0:128, :], w1c[128:256, :], np.eye(128, dtype=np_dts),
         w2c.reshape(128, 1)], axis=1).astype(np_dts)
    ck32 = np.concatenate([b1c, b2c, blkid], axis=1).astype(np.float32)

    # out row for graph gid: core*ngroups*gt + group*gt + slot
    pos = np.zeros(g_total, np.int64)
    in_maps = []
    for core in range(n_cores):
        xa = np.zeros((ngroups * C, XW), np.float32)
        crel_flat = np.full(ngroups * C, -1.0, np.float32)
        for g in range(ngroups):
            b = core * ngroups + g
            off = g * C
            for s, gid in enumerate(members[b]):
                s0, n = int(starts[gid]), int(counts[gid])
                xa[off:off + n, :HID] = x[s0:s0 + n]
                crel_flat[off:off + n] = float(s)
                pos[gid] = (core * ngroups + g) * gt + s
                off += n
        xa[:, ONES_COL] = 1.0
        xtfull = np.ascontiguousarray(xa[:, :HID].T).astype(np_xt)
        # pack x^T per chunk (both hidden halves back to back, contiguous)
        xtw = sched[-1][5] + sched[-1][4] * 256
        xt = np.zeros((128, max(xtw, 256)), np_xt)
        for g, c0, nt, tk, nl, xo in sched:
            n0 = (g * ntpg + c0) * 128
            xt[:, xo:xo + nl * 128] = xtfull[0:128, n0:n0 + nl * 128]
            xt[:, xo + nl * 128:xo + nl * 256] = xtfull[128:256, n0:n0 + nl * 128]
        # partition-major swizzle: xa_swz[p, t*XW + d] = xa[t*128 + p, d]
        xa_swz = np.ascontiguousarray(
            xa.astype(np_dts).reshape(T, 128, XW).transpose(1, 0, 2)
        ).reshape(128, T * XW)
        in_maps.append({
            "xa": xa_swz,
            "xt": xt,
            "crel": np.ascontiguousarray(crel_flat.reshape(T, 128).T)
                      .astype(np_dts),
            "ck16": ck16, "ck32": ck32, "iota": iota,
        })
    return in_maps, ntpg, pos


# ================================================================ entry
LAST_RESULTS = None


def kernel(x, batch, W1, b1, W2, b2):
    global LAST_RESULTS
    from concourse.bass_utils import run_bass_kernel_spmd

    in_maps, ntpg, pos = prepare_shards(x, batch, W1, b1, W2, b2)
    key = (ntpg, USE_FP16, USE_XT8, TK)
    if key not in _nc_cache:
        _nc_cache[key] = build_bass(ntpg)
    nc = _nc_cache[key]
    trace = os.environ.get("KERNEL_TRACE", "0") == "1"
    res = run_bass_kernel_spmd(nc, in_maps, core_ids=list(range(N_CORES)),
                               trace=trace)
    LAST_RESULTS = res
    pooled = np.concatenate([r["out"] for r in res.results], axis=0)
    return pooled[pos].astype(np.float32)
